# revision 1
# baseline (speedup 1.0000x reference)
"""Trainium2 Bass kernel for two-stage pooled-query attention.

Problem (hardcoded):
    B=32, N=577, C=1024, H=16 heads, d=64, pooled queries 8x8 (3x3 mean over
    24x24 grid of non-cls tokens).
    qkv = X @ W_qkv.T ; pool Xq -> Qp ; s1 = softmax(Qp*s @ K^T) @ V ;
    s2 = softmax(Xq*s @ Qp^T) @ s1 ; out = s2 @ W_proj.T + b_proj

Strategy: pure data-parallel over batch across 8 NeuronCores (4 batches per
core, no collectives). All matmuls run bf16 with fp32 PSUM accumulation.
Layout is chosen so every contraction sits on SBUF partitions, and every
matmul uses a full K=128 contraction (partial-K matmuls misbehave on HW):
  - X arrives pre-transposed as XT [k, n] from the host (layout prep,
    like the weights), zero-padded to 640 tokens.
  - QKV GEMM emits Xq/Xk transposed [c, n] and V natural [n, c].
  - Pooling is a strided-AP reduce over XqT columns (exact 3x3 mean).
  - Per head-pair, pooled queries go into a block-diagonal [128, 128] lhsT
    so both heads' scores come from one K=128 matmul.
  - Stage-1 Qd is computed as a full [2q, 2dv] pair product; only the
    per-head diagonal blocks are kept (and softmax-normalized) on evict.
  - Attention output is produced transposed [c, n], which is exactly the
    lhsT layout the output projection needs; bias is pre-broadcast once
    and added during the output evict copy.
"""

import os
import sys

import numpy as np

sys.path.insert(0, "/opt/trn_rl_repo")

import ml_dtypes  # noqa: E402

import concourse.tile as tile  # noqa: E402
from concourse import bacc, mybir  # noqa: E402
from concourse.bass_utils import run_bass_kernel_spmd  # noqa: E402
from concourse.masks import make_identity  # noqa: E402

B, N, C = 32, 577, 1024
H, D = 16, 64
SCALE = D ** -0.5
N_CORES = 8
NB = B // N_CORES  # batches per core

BF16 = mybir.dt.bfloat16
F32 = mybir.dt.float32

# token chunks of 577 = 4*128 + 65
TOK = [(0, 128), (128, 128), (256, 128), (384, 128), (512, 65)]
# free-dim chunks of 577 for wide matmuls / psum banks
NF = [(0, 320), (320, 257)]
EXP = mybir.ActivationFunctionType.Exp


def build_program(nb: int = NB):
    nc = bacc.Bacc("TRN2", target_bir_lowering=False, debug=False)

    x_d = nc.dram_tensor("xt", [nb, C, 640], BF16, kind="ExternalInput")
    wqkvt_d = nc.dram_tensor("wqkvt", [C, 3 * C], BF16, kind="ExternalInput")
    wprojt_d = nc.dram_tensor("wprojt", [C, C], BF16, kind="ExternalInput")
    wbias_d = nc.dram_tensor("wbias", [1, C], BF16, kind="ExternalInput")
    out_d = nc.dram_tensor("out", [nb, N, C], F32, kind="ExternalOutput")

    with tile.TileContext(nc) as tc:
        const_pool = tc.alloc_tile_pool(name="const", bufs=1)
        w_pool = tc.alloc_tile_pool(name="w", bufs=1)
        sb = tc.alloc_tile_pool(name="sb", bufs=2)
        ps_big = tc.alloc_tile_pool(name="ps_big", bufs=5, space="PSUM")
        ps_small = tc.alloc_tile_pool(name="ps_small", bufs=3, space="PSUM")

        ident = const_pool.tile([128, 128], BF16, tag="ident")
        make_identity(nc, ident[:])
        ones = const_pool.tile([1, 128], BF16, tag="ones")
        nc.gpsimd.memset(ones[:], 1.0)

        # first batch's XT goes out before the (much larger) weight DMAs so
        # the QKV gemm can start immediately; weights stream behind.
        XT0 = []
        for j in range(8):
            xtj = sb.tile([128, 640], BF16, tag=f"xt{j}", bufs=2)
            nc.sync.dma_start(xtj[:], x_d[0, 128 * j : 128 * (j + 1), :])
            XT0.append(xtj)

        # resident weights, streamed in 512-col chunks: q/k columns first so
        # the QKV gemm can start as soon as its first chunks land, V and
        # proj weights trail behind.
        wq = []
        for j in range(8):
            wqt = w_pool.tile([128, 3 * C], BF16, tag=f"wq{j}")
            wq.append(wqt)
        for blk in range(4):
            for j in range(8):
                cs = slice(512 * blk, 512 * (blk + 1))
                nc.sync.dma_start(wq[j][:, cs], wqkvt_d[128 * j : 128 * (j + 1), cs])
        for blk in range(4, 6):
            for j in range(8):
                cs = slice(512 * blk, 512 * (blk + 1))
                nc.sync.dma_start(wq[j][:, cs], wqkvt_d[128 * j : 128 * (j + 1), cs])
        wp = []
        for j in range(8):
            t = w_pool.tile([128, C], BF16, tag=f"wp{j}")
            nc.sync.dma_start(t[:], wprojt_d[128 * j : 128 * (j + 1), :])
            wp.append(t)
        wb = w_pool.tile([1, C], BF16, tag="wb")
        nc.sync.dma_start(wb[:], wbias_d[:])

        # bias broadcast [128, 1024]; built lazily (first use is phase 8)
        bias = const_pool.tile([128, C], BF16, tag="bias")
        bias_built = [False]

        def build_bias():
            if bias_built[0]:
                return
            bias_built[0] = True
            for half in range(2):
                cs = slice(512 * half, 512 * (half + 1))
                bps = ps_big.tile([128, 512], F32, tag="pbig")
                nc.tensor.matmul(
                    bps[:], ones[0:1, :], wb[0:1, cs], start=True, stop=True
                )
                nc.any.tensor_copy(bias[:, cs], bps[:])

        repeat = int(os.environ.get("KERNEL_REPEAT", "1"))
        for b in [bb for _ in range(repeat) for bb in range(nb)]:
            # ---- Phase 1: XT [k, n] arrives pre-transposed and zero-padded
            # from the host (layout prep, like the weights) ----
            if b == 0 and XT0 is not None:
                XT, XT0 = XT0, None
            else:
                XT = []
                for j in range(8):
                    xtj = sb.tile([128, 640], BF16, tag=f"xt{j}", bufs=2)
                    nc.sync.dma_start(xtj[:], x_d[b, 128 * j : 128 * (j + 1), :])
                    XT.append(xtj)

            # ---- Phase 2: QKV gemm, q/k parts transposed: qkT [c, n] ----
            qkT = []
            for cc in range(16):
                qt = sb.tile([128, 640], BF16, tag=f"qkt{cc}", bufs=1)
                for ci, (n0, nw) in enumerate(NF):
                    ps = ps_big.tile([128, nw], F32, tag="pbig")
                    for j in range(8):
                        nc.tensor.matmul(
                            ps[:],
                            wq[j][:, 128 * cc : 128 * (cc + 1)],
                            XT[j][:, n0 : n0 + nw],
                            start=(j == 0),
                            stop=(j == 7),
                        )
                    if (cc + ci) % 2 == 0:
                        nc.vector.tensor_copy(qt[:, n0 : n0 + nw], ps[:])
                    else:
                        nc.scalar.copy(qt[:, n0 : n0 + nw], ps[:])
                nc.any.memset(qt[:, 577:640], 0.0)
                qkT.append(qt)

            # ---- Phase 3: V part natural layout [n, c]; tail rows zero ----
            V = []
            for t, (toff, rows) in enumerate(TOK):
                vt = sb.tile([128, C], BF16, tag=f"v{t}", bufs=1)
                lo = toff if rows == 128 else 512
                for h2 in range(2):
                    ps = ps_big.tile([128, 512], F32, tag="pbig")
                    for j in range(8):
                        nc.tensor.matmul(
                            ps[:],
                            XT[j][:, lo : lo + 128],
                            wq[j][:, 2048 + 512 * h2 : 2048 + 512 * (h2 + 1)],
                            start=(j == 0),
                            stop=(j == 7),
                        )
                    nc.any.tensor_copy(vt[:, 512 * h2 : 512 * (h2 + 1)], ps[:])
                V.append(vt)

            # ---- Phase 4: pooled queries, block-diag QpBD per pair ----
            QpBD = []
            for j in range(8):
                qsum = sb.tile([128, 64], F32, tag="qsum", bufs=3)
                view = qkT[j][:, 0:576].rearrange(
                    "p (pr dr pc dc) -> p pr pc dr dc", pr=8, dr=3, pc=8, dc=3
                )
                nc.vector.reduce_sum(qsum[:], view, axis=mybir.AxisListType.XY)
                qp = sb.tile([128, 128], BF16, tag=f"qp{j}")
                nc.any.memset(qp[:], 0.0)
                nc.scalar.mul(qp[0:64, 0:64], qsum[0:64, :], SCALE / 9.0)
                nc.scalar.mul(qp[64:128, 64:128], qsum[64:128, :], SCALE / 9.0)
                QpBD.append(qp)

            # ---- Phase 6: stage-2 scores + exp + normalize (emitted
            # per-octet; octet 0 is interleaved into phase 5 so the softmax
            # epilogue engines start early) ----
            def s2_chain(oc):
                for t, (toff, rows) in enumerate(TOK):
                    lo = toff if rows == 128 else 512
                    ps = ps_big.tile([128, 512], F32, tag="pbig")
                    for pz in range(4):
                        pp = 4 * oc + pz
                        nc.tensor.matmul(
                            ps[:, 128 * pz : 128 * (pz + 1)],
                            qkT[pp][:, lo : lo + 128],
                            QpBD[pp][:],
                            start=True,
                            stop=True,
                        )
                    s2e = sb.tile([128, 512], F32, tag="s2e", bufs=4)
                    nc.scalar.activation(s2e[0:rows, :], ps[0:rows, :], EXP)
                    s2s = sb.tile([128, 8], F32, tag="s2s", bufs=4)
                    nc.vector.reduce_sum(
                        s2s[0:rows, :],
                        s2e[0:rows, :].rearrange("p (h q) -> p h q", q=64),
                        axis=mybir.AxisListType.X,
                    )
                    r2 = sb.tile([128, 8], F32, tag="r2", bufs=4)
                    nc.vector.reciprocal(r2[0:rows, :], s2s[0:rows, :])
                    a2 = sb.tile([128, 512], BF16, tag=f"a2n{t}_{oc}", bufs=1)
                    for pz in range(4):
                        eng = nc.vector if pz == 0 else nc.gpsimd
                        zs = slice(128 * pz, 128 * (pz + 1))
                        eng.tensor_tensor(
                            a2[0:rows, zs].rearrange("p (h q) -> p h q", q=64),
                            s2e[0:rows, zs].rearrange("p (h q) -> p h q", q=64),
                            r2[0:rows, 2 * pz : 2 * pz + 2]
                            .unsqueeze(2)
                            .broadcast_to((rows, 2, 64)),
                            op=mybir.AluOpType.mult,
                        )
                    A2n[t][oc] = a2


            # ---- Phase 5: stage-1 attention per head-pair (1-pair skew so
            # the PE never waits on the exp of the pair it just scored) ----
            QdBD = []
            s1_state = {}

            def s1_score(pp):
                a1 = sb.tile([128, 640], BF16, tag="a1", bufs=3)
                esum = sb.tile([128, 2], F32, tag="esum", bufs=3)
                for ci, (n0, nw) in enumerate(NF):
                    ps = ps_big.tile([128, nw], F32, tag="pbig")
                    nc.tensor.matmul(
                        ps[:],
                        QpBD[pp][:],
                        qkT[8 + pp][:, n0 : n0 + nw],
                        start=True,
                        stop=True,
                    )
                    nc.scalar.activation(
                        a1[:, n0 : n0 + nw],
                        ps[:],
                        EXP,
                        accum_out=esum[:, ci : ci + 1],
                    )
                nc.any.memset(a1[:, 577:640], 0.0)
                ssum = sb.tile([128, 1], F32, tag="ssum", bufs=3)
                nc.vector.tensor_add(ssum[:], esum[:, 0:1], esum[:, 1:2])
                r1 = sb.tile([128, 1], F32, tag="r1", bufs=4)
                nc.vector.reciprocal(r1[:], ssum[:])
                s1_state[pp] = (a1, r1)

            a1t_state = {}

            def s1_transpose(pp):
                a1, r1 = s1_state.pop(pp)
                # transpose A1 -> [n, 2q], full 128-wide tail (zero padded)
                pa = ps_big.tile([128, 512], BF16, tag="pbig")
                for t in range(4):
                    nc.tensor.transpose(
                        pa[:, 128 * t : 128 * (t + 1)],
                        a1[:, 128 * t : 128 * (t + 1)],
                        ident[:],
                    )
                pb = ps_small.tile([128, 128], BF16, tag="psmall")
                nc.tensor.transpose(pb[:], a1[:, 512:640], ident[:])
                a1t = sb.tile([128, 640], BF16, tag="a1t", bufs=3)
                nc.any.tensor_copy(a1t[:, 0:512], pa[:])
                nc.any.tensor_copy(a1t[:, 512:640], pb[:])
                a1t_state[pp] = (a1t, r1)

            def s1_qd(pp):
                a1t, r1 = a1t_state.pop(pp)
                # Qd pair product [2q, 2dv]; keep diagonal blocks, scaled by r1
                qd_ps = ps_small.tile([128, 128], F32, tag="psmall")
                for t in range(5):
                    nc.tensor.matmul(
                        qd_ps[:],
                        a1t[:, 128 * t : 128 * (t + 1)],
                        V[t][:, 128 * pp : 128 * (pp + 1)],
                        start=(t == 0),
                        stop=(t == 4),
                    )
                qd = sb.tile([128, 128], BF16, tag=f"qd{pp}")
                nc.any.memset(qd[:], 0.0)
                nc.vector.tensor_scalar_mul(
                    qd[0:64, 0:64], qd_ps[0:64, 0:64], r1[0:64, 0:1]
                )
                nc.vector.tensor_scalar_mul(
                    qd[64:128, 64:128], qd_ps[64:128, 64:128], r1[64:128, 0:1]
                )
                QdBD.append(qd)

            A2n = [[None, None] for _ in range(5)]
            for pp in range(8):
                s1_score(pp)
                if pp > 0:
                    s1_transpose(pp - 1)
                if pp > 1:
                    s1_qd(pp - 2)
                if pp == 2:
                    s2_chain(0)
                if pp == 6:
                    s2_chain(1)
            s1_transpose(7)
            s1_qd(6)
            s1_qd(7)


            # ---- Phase 7: transpose A2, outT = QdBD.T @ A2T -> [c, n],
            # with outT matmuls lagging one pair behind the transposes ----
            outT = []
            a2t_state = {}

            def a2_transpose(pp):
                oc, sl = pp // 4, 128 * (pp % 4)
                pa = ps_big.tile([128, 512], BF16, tag="pbig")
                for t in range(4):
                    nc.tensor.transpose(
                        pa[:, 128 * t : 128 * (t + 1)],
                        A2n[t][oc][:, sl : sl + 128],
                        ident[:],
                    )
                pb = ps_small.tile([128, 128], BF16, tag="psmall")
                nc.tensor.transpose(
                    pb[:, 0:65], A2n[4][oc][0:65, sl : sl + 128], ident[0:65, 0:65]
                )
                a2t = sb.tile([128, 640], BF16, tag="a2t", bufs=4)
                nc.any.tensor_copy(a2t[:, 0:512], pa[:])
                nc.any.tensor_copy(a2t[:, 512:577], pb[:, 0:65])
                nc.any.memset(a2t[:, 577:640], 0.0)
                a2t_state[pp] = a2t

            def out_mm(pp):
                a2t = a2t_state.pop(pp)
                oa = ps_big.tile([128, 512], F32, tag="pbig")
                ob = ps_big.tile([128, 128], F32, tag="pbig")
                nc.tensor.matmul(
                    oa[:], QdBD[pp][:], a2t[:, 0:512], start=True, stop=True
                )
                nc.tensor.matmul(
                    ob[:], QdBD[pp][:], a2t[:, 512:640], start=True, stop=True
                )
                ot = sb.tile([128, 640], BF16, tag=f"ot{pp}", bufs=1)
                nc.any.tensor_copy(ot[:, 0:512], oa[:])
                nc.any.tensor_copy(ot[:, 512:640], ob[:])
                outT.append(ot)

            a2_transpose(0)
            a2_transpose(1)
            for pp in range(2, 8):
                a2_transpose(pp)
                out_mm(pp - 2)
            out_mm(6)
            out_mm(7)

            # ---- Phase 8: output projection + bias, then to DRAM ----
            build_bias()
            for t, (toff, rows) in enumerate(TOK):
                lo = toff if rows == 128 else 512
                for half in range(2):
                    cs = slice(512 * half, 512 * (half + 1))
                    ps = ps_big.tile([128, 512], F32, tag="pbig")
                    for cc in range(8):
                        nc.tensor.matmul(
                            ps[:],
                            outT[cc][:, lo : lo + 128],
                            wp[cc][:, cs],
                            start=(cc == 0),
                            stop=(cc == 7),
                        )
                    y = sb.tile([128, 512], F32, tag="y", bufs=3)
                    nc.vector.tensor_add(y[0:rows, :], ps[0:rows, :], bias[0:rows, cs])
                    nc.sync.dma_start(out_d[b, toff : toff + rows, cs], y[0:rows, :])

        for p in (ps_small, ps_big, sb, w_pool, const_pool):
            p.release()

    nc.compile()
    return nc


_NC_CACHE = {}


def _get_nc(nb: int = NB):
    if nb not in _NC_CACHE:
        _NC_CACHE[nb] = build_program(nb)
    return _NC_CACHE[nb]


def kernel(X, W_qkv, W_proj, b_proj, layer_idx=None):
    assert X.shape == (B, N, C)
    nc = _get_nc(NB)
    xt = np.zeros((B, C, 640), dtype=np.float32)
    xt[:, :, :N] = np.asarray(X, dtype=np.float32).transpose(0, 2, 1)
    xb = xt.astype(ml_dtypes.bfloat16)
    wqkvt = np.ascontiguousarray(np.asarray(W_qkv, dtype=np.float32).T).astype(
        ml_dtypes.bfloat16
    )
    wprojt = np.ascontiguousarray(np.asarray(W_proj, dtype=np.float32).T).astype(
        ml_dtypes.bfloat16
    )
    wbias = np.asarray(b_proj, dtype=np.float32).reshape(1, C).astype(
        ml_dtypes.bfloat16
    )
    in_maps = [
        {
            "xt": xb[NB * i : NB * (i + 1)],
            "wqkvt": wqkvt,
            "wprojt": wprojt,
            "wbias": wbias,
        }
        for i in range(N_CORES)
    ]
    res = run_bass_kernel_spmd(nc, in_maps, core_ids=list(range(N_CORES)))
    out = np.concatenate([res.results[i]["out"] for i in range(N_CORES)], axis=0)
    return out.astype(np.float32)


if __name__ == "__main__":
    rng = np.random.default_rng(0)
    X = rng.standard_normal((B, N, C), dtype=np.float32)
    W_qkv = rng.standard_normal((3 * C, C), dtype=np.float32) * C**-0.5
    W_proj = rng.standard_normal((C, C), dtype=np.float32) * C**-0.5
    b_proj = np.zeros(C, dtype=np.float32)
    out = kernel(X, W_qkv, W_proj, b_proj, 1)
    print(out.shape, out.dtype)



# revision 8
# speedup vs baseline: 1.1335x; 1.1335x over previous
"""Trainium2 Bass kernel for two-stage pooled-query attention.

Problem (hardcoded):
    B=32, N=577, C=1024, H=16 heads, d=64, pooled queries 8x8 (3x3 mean over
    24x24 grid of non-cls tokens).
    qkv = X @ W_qkv.T ; pool Xq -> Qp ; s1 = softmax(Qp*s @ K^T) @ V ;
    s2 = softmax(Xq*s @ Qp^T) @ s1 ; out = s2 @ W_proj.T + b_proj

Strategy: pure data-parallel over batch across 8 NeuronCores (4 batches per
core, no collectives). The attention middle runs bf16 with fp32 PSUM
accumulation; the big QKV GEMM runs fp8 DoubleRow (K=256 per instruction at
0.5 cycles/row) with a 3-term hi/lo hybrid split to hold accuracy:
  X @ W ~= X_hi@W_hi + X_lo@W_hi + X_hi@W_lo, each operand split on the host
  into an e4m3 value plus an e4m3 residual (W_qkv pre-scaled by 32 so its
  residual clears the fp8 subnormal floor; the 32 cancels through the pooled
  query scale and W_proj/32).
Layout is chosen so every contraction sits on SBUF partitions, and every
matmul uses a full K=128 contraction (partial-K matmuls misbehave on HW):
  - X arrives pre-transposed as XT [k, n] from the host (layout prep,
    like the weights), zero-padded to 640 tokens, shipped as fp8 hi/lo
    chunk-pair tiles [128, 2, 640] for DoubleRow.
  - QKV GEMM emits Xq/Xk transposed [c, n] and V natural [n, c].
  - Pooling is a strided-AP reduce over XqT columns (exact 3x3 mean).
  - Per head-pair, pooled queries go into a block-diagonal [128, 128] lhsT
    so both heads' scores come from one K=128 matmul.
  - Stage-1 Qd is computed as a full [2q, 2dv] pair product; only the
    per-head diagonal blocks are kept (and softmax-normalized) on evict.
  - Attention output is produced transposed [c, n], which is exactly the
    lhsT layout the output projection needs; bias is pre-broadcast once
    and added during the output evict copy.
"""

import os
import sys

import numpy as np

sys.path.insert(0, "/opt/trn_rl_repo")

import ml_dtypes  # noqa: E402

import concourse.tile as tile  # noqa: E402
from concourse import bacc, mybir  # noqa: E402
from concourse.bass_utils import run_bass_kernel_spmd  # noqa: E402
from concourse.masks import make_identity  # noqa: E402

B, N, C = 32, 577, 1024
H, D = 16, 64
SCALE = D ** -0.5
N_CORES = 8
NB = B // N_CORES  # batches per core

BF16 = mybir.dt.bfloat16
F32 = mybir.dt.float32
FP8 = mybir.dt.float8e4
DR = mybir.MatmulPerfMode.DoubleRow
WSCALE = 32.0  # host pre-scale on W_qkv^T (keeps fp8 residuals normal)

# token chunks of 577 = 4*128 + 65
TOK = [(0, 128), (128, 128), (256, 128), (384, 128), (512, 65)]
# free-dim chunks of 577 for wide matmuls / psum banks
NF = [(0, 320), (320, 257)]
EXP = mybir.ActivationFunctionType.Exp


def build_program(nb: int = NB):
    nc = bacc.Bacc("TRN2", target_bir_lowering=False, debug=False)

    xhi_d = nc.dram_tensor("xhi", [nb, C, 640], FP8, kind="ExternalInput")
    xlo_d = nc.dram_tensor("xlo", [nb, C, 640], FP8, kind="ExternalInput")
    whi_d = nc.dram_tensor("whi", [C, 3 * C], FP8, kind="ExternalInput")
    wlo_d = nc.dram_tensor("wlo", [C, 3 * C], FP8, kind="ExternalInput")
    wprojt_d = nc.dram_tensor("wprojt", [C, C], BF16, kind="ExternalInput")
    wbias_d = nc.dram_tensor("wbias", [1, C], BF16, kind="ExternalInput")
    out_d = nc.dram_tensor("out", [nb, N, C], F32, kind="ExternalOutput")

    with tile.TileContext(nc) as tc:
        const_pool = tc.alloc_tile_pool(name="const", bufs=1)
        w_pool = tc.alloc_tile_pool(name="w", bufs=1)
        sb = tc.alloc_tile_pool(name="sb", bufs=2)
        ps_big = tc.alloc_tile_pool(name="ps_big", bufs=5, space="PSUM")
        ps_small = tc.alloc_tile_pool(name="ps_small", bufs=3, space="PSUM")

        ident = const_pool.tile([128, 128], BF16, tag="ident")
        make_identity(nc, ident[:])
        ones = const_pool.tile([1, 128], BF16, tag="ones")
        nc.gpsimd.memset(ones[:], 1.0)

        # first batch's X goes out before the (much larger) weight DMAs so
        # the QKV gemm can start immediately; weights stream behind. X and W
        # ship as fp8 hi/lo chunk-pair tiles [128, 2, cols] (dim 1 = the two
        # K=128 sub-chunks one DoubleRow matmul contracts).
        def load_x(b):
            tiles = []
            for c2 in range(4):
                src = slice(256 * c2, 256 * (c2 + 1))
                xh = sb.tile([128, 2, 640], FP8, tag=f"xh{c2}", bufs=2)
                nc.sync.dma_start(
                    xh[:], xhi_d[b, src, :].rearrange("(i p) n -> p i n", i=2)
                )
                xl = sb.tile([128, 2, 640], FP8, tag=f"xl{c2}", bufs=2)
                nc.sync.dma_start(
                    xl[:], xlo_d[b, src, :].rearrange("(i p) n -> p i n", i=2)
                )
                tiles.append((xh, xl))
            return tiles

        XT0 = load_x(0)

        # resident weights, streamed in 512-col chunks: q/k columns first so
        # the QKV gemm can start as soon as its first chunks land, V and
        # proj weights trail behind.
        wh, wl = [], []
        for c2 in range(4):
            wht = w_pool.tile([128, 2, 3 * C], FP8, tag=f"wh{c2}")
            wh.append(wht)
            wlt = w_pool.tile([128, 2, 3 * C], FP8, tag=f"wl{c2}")
            wl.append(wlt)
        for blk in range(6):
            cs = slice(512 * blk, 512 * (blk + 1))
            for c2 in range(4):
                src = slice(256 * c2, 256 * (c2 + 1))
                nc.sync.dma_start(
                    wh[c2][:, :, cs],
                    whi_d[src, cs].rearrange("(i p) n -> p i n", i=2),
                )
                nc.sync.dma_start(
                    wl[c2][:, :, cs],
                    wlo_d[src, cs].rearrange("(i p) n -> p i n", i=2),
                )
        wp = []
        for j in range(8):
            t = w_pool.tile([128, C], BF16, tag=f"wp{j}")
            nc.sync.dma_start(t[:], wprojt_d[128 * j : 128 * (j + 1), :])
            wp.append(t)
        wb = w_pool.tile([1, C], BF16, tag="wb")
        nc.sync.dma_start(wb[:], wbias_d[:])

        # bias broadcast [128, 1024]; built lazily (first use is phase 8)
        bias = const_pool.tile([128, C], BF16, tag="bias")
        bias_built = [False]

        def build_bias():
            if bias_built[0]:
                return
            bias_built[0] = True
            for half in range(2):
                cs = slice(512 * half, 512 * (half + 1))
                bps = ps_big.tile([128, 512], F32, tag="pbig")
                nc.tensor.matmul(
                    bps[:], ones[0:1, :], wb[0:1, cs], start=True, stop=True
                )
                nc.any.tensor_copy(bias[:, cs], bps[:])

        repeat = int(os.environ.get("KERNEL_REPEAT", "1"))
        for b in [bb for _ in range(repeat) for bb in range(nb)]:
            # ---- Phase 1: X [k, n] arrives pre-transposed, zero-padded,
            # fp8 hi/lo split from the host (layout prep, like the weights) --
            if b == 0 and XT0 is not None:
                XT, XT0 = XT0, None
            else:
                XT = load_x(b)

            # ---- Phase 2: QKV gemm (fp8 DoubleRow, 3-term hybrid),
            # q/k parts transposed: qkT [c, n] = 32 * true ----
            qkT = []
            for cc in range(16):
                qt = sb.tile([128, 640], BF16, tag=f"qkt{cc}", bufs=1)
                csl = slice(128 * cc, 128 * (cc + 1))
                for ci, (n0, nw) in enumerate(NF):
                    ps = ps_big.tile([128, nw], F32, tag="pbig")
                    terms = (
                        [(wh[c2], XT[c2][0]) for c2 in range(4)]
                        + [(wh[c2], XT[c2][1]) for c2 in range(4)]
                        + [(wl[c2], XT[c2][0]) for c2 in range(4)]
                    )
                    for ti, (wt, xt_) in enumerate(terms):
                        nc.tensor.matmul(
                            ps[:],
                            wt[:, :, csl],
                            xt_[:, :, n0 : n0 + nw],
                            start=(ti == 0),
                            stop=(ti == 11),
                            perf_mode=DR,
                        )
                    if (cc + ci) % 2 == 0:
                        nc.vector.tensor_copy(qt[:, n0 : n0 + nw], ps[:])
                    else:
                        nc.scalar.copy(qt[:, n0 : n0 + nw], ps[:])
                nc.any.memset(qt[:, 577:640], 0.0)
                qkT.append(qt)

            # ---- Phase 3: V part natural layout [n, c] = 32 * true;
            # tail rows zero (X pad columns are zero in hi and lo) ----
            V = []
            for t, (toff, rows) in enumerate(TOK):
                vt = sb.tile([128, C], BF16, tag=f"v{t}", bufs=1)
                lo = toff if rows == 128 else 512
                for h2 in range(2):
                    ps = ps_big.tile([128, 512], F32, tag="pbig")
                    cs = slice(2048 + 512 * h2, 2048 + 512 * (h2 + 1))
                    terms = (
                        [(XT[c2][0], wh[c2]) for c2 in range(4)]
                        + [(XT[c2][1], wh[c2]) for c2 in range(4)]
                        + [(XT[c2][0], wl[c2]) for c2 in range(4)]
                    )
                    for ti, (xt_, wt) in enumerate(terms):
                        nc.tensor.matmul(
                            ps[:],
                            xt_[:, :, lo : lo + 128],
                            wt[:, :, cs],
                            start=(ti == 0),
                            stop=(ti == 11),
                            perf_mode=DR,
                        )
                    nc.any.tensor_copy(vt[:, 512 * h2 : 512 * (h2 + 1)], ps[:])
                V.append(vt)

            # ---- Phase 4: pooled queries, block-diag QpBD per pair ----
            QpBD = []
            for j in range(8):
                qsum = sb.tile([128, 64], F32, tag="qsum", bufs=3)
                view = qkT[j][:, 0:576].rearrange(
                    "p (pr dr pc dc) -> p pr pc dr dc", pr=8, dr=3, pc=8, dc=3
                )
                nc.vector.reduce_sum(qsum[:], view, axis=mybir.AxisListType.XY)
                # qkT carries 32x; qp = SCALE * Qp_true / 32 makes both score
                # matmuls (qp @ qkT_k and qkT_q @ qp) come out exactly right.
                alpha = SCALE / (9.0 * WSCALE * WSCALE)
                qp = sb.tile([128, 128], BF16, tag=f"qp{j}")
                nc.any.memset(qp[:], 0.0)
                nc.scalar.mul(qp[0:64, 0:64], qsum[0:64, :], alpha)
                nc.scalar.mul(qp[64:128, 64:128], qsum[64:128, :], alpha)
                QpBD.append(qp)

            # ---- Phase 6: stage-2 scores + exp + normalize (emitted
            # per-octet; octet 0 is interleaved into phase 5 so the softmax
            # epilogue engines start early) ----
            def s2_chain(oc):
                for t, (toff, rows) in enumerate(TOK):
                    lo = toff if rows == 128 else 512
                    ps = ps_big.tile([128, 512], F32, tag="pbig")
                    for pz in range(4):
                        pp = 4 * oc + pz
                        nc.tensor.matmul(
                            ps[:, 128 * pz : 128 * (pz + 1)],
                            qkT[pp][:, lo : lo + 128],
                            QpBD[pp][:],
                            start=True,
                            stop=True,
                        )
                    s2e = sb.tile([128, 512], F32, tag="s2e", bufs=4)
                    nc.scalar.activation(s2e[0:rows, :], ps[0:rows, :], EXP)
                    s2s = sb.tile([128, 8], F32, tag="s2s", bufs=4)
                    nc.vector.reduce_sum(
                        s2s[0:rows, :],
                        s2e[0:rows, :].rearrange("p (h q) -> p h q", q=64),
                        axis=mybir.AxisListType.X,
                    )
                    r2 = sb.tile([128, 8], F32, tag="r2", bufs=4)
                    nc.vector.reciprocal(r2[0:rows, :], s2s[0:rows, :])
                    a2 = sb.tile([128, 512], BF16, tag=f"a2n{t}_{oc}", bufs=1)
                    for pz in range(4):
                        eng = nc.vector if pz == 0 else nc.gpsimd
                        zs = slice(128 * pz, 128 * (pz + 1))
                        eng.tensor_tensor(
                            a2[0:rows, zs].rearrange("p (h q) -> p h q", q=64),
                            s2e[0:rows, zs].rearrange("p (h q) -> p h q", q=64),
                            r2[0:rows, 2 * pz : 2 * pz + 2]
                            .unsqueeze(2)
                            .broadcast_to((rows, 2, 64)),
                            op=mybir.AluOpType.mult,
                        )
                    A2n[t][oc] = a2


            # ---- Phase 5: stage-1 attention per head-pair (1-pair skew so
            # the PE never waits on the exp of the pair it just scored) ----
            QdBD = []
            s1_state = {}

            def s1_score(pp):
                a1 = sb.tile([128, 640], BF16, tag="a1", bufs=3)
                esum = sb.tile([128, 2], F32, tag="esum", bufs=3)
                for ci, (n0, nw) in enumerate(NF):
                    ps = ps_big.tile([128, nw], F32, tag="pbig")
                    nc.tensor.matmul(
                        ps[:],
                        QpBD[pp][:],
                        qkT[8 + pp][:, n0 : n0 + nw],
                        start=True,
                        stop=True,
                    )
                    nc.scalar.activation(
                        a1[:, n0 : n0 + nw],
                        ps[:],
                        EXP,
                        accum_out=esum[:, ci : ci + 1],
                    )
                nc.any.memset(a1[:, 577:640], 0.0)
                ssum = sb.tile([128, 1], F32, tag="ssum", bufs=3)
                nc.vector.tensor_add(ssum[:], esum[:, 0:1], esum[:, 1:2])
                r1 = sb.tile([128, 1], F32, tag="r1", bufs=4)
                nc.vector.reciprocal(r1[:], ssum[:])
                s1_state[pp] = (a1, r1)

            a1t_state = {}

            def s1_transpose(pp):
                a1, r1 = s1_state.pop(pp)
                # transpose A1 -> [n, 2q], full 128-wide tail (zero padded)
                pa = ps_big.tile([128, 512], BF16, tag="pbig")
                for t in range(4):
                    nc.tensor.transpose(
                        pa[:, 128 * t : 128 * (t + 1)],
                        a1[:, 128 * t : 128 * (t + 1)],
                        ident[:],
                    )
                pb = ps_small.tile([128, 128], BF16, tag="psmall")
                nc.tensor.transpose(pb[:], a1[:, 512:640], ident[:])
                a1t = sb.tile([128, 640], BF16, tag="a1t", bufs=3)
                nc.any.tensor_copy(a1t[:, 0:512], pa[:])
                nc.any.tensor_copy(a1t[:, 512:640], pb[:])
                a1t_state[pp] = (a1t, r1)

            def s1_qd(pp):
                a1t, r1 = a1t_state.pop(pp)
                # Qd pair product [2q, 2dv]; keep diagonal blocks, scaled by r1
                qd_ps = ps_small.tile([128, 128], F32, tag="psmall")
                for t in range(5):
                    nc.tensor.matmul(
                        qd_ps[:],
                        a1t[:, 128 * t : 128 * (t + 1)],
                        V[t][:, 128 * pp : 128 * (pp + 1)],
                        start=(t == 0),
                        stop=(t == 4),
                    )
                qd = sb.tile([128, 128], BF16, tag=f"qd{pp}")
                nc.any.memset(qd[:], 0.0)
                nc.vector.tensor_scalar_mul(
                    qd[0:64, 0:64], qd_ps[0:64, 0:64], r1[0:64, 0:1]
                )
                nc.vector.tensor_scalar_mul(
                    qd[64:128, 64:128], qd_ps[64:128, 64:128], r1[64:128, 0:1]
                )
                QdBD.append(qd)

            A2n = [[None, None] for _ in range(5)]
            for pp in range(8):
                s1_score(pp)
                if pp > 0:
                    s1_transpose(pp - 1)
                if pp > 1:
                    s1_qd(pp - 2)
                if pp == 2:
                    s2_chain(0)
                if pp == 6:
                    s2_chain(1)
            s1_transpose(7)
            s1_qd(6)
            s1_qd(7)


            # ---- Phase 7: transpose A2, outT = QdBD.T @ A2T -> [c, n],
            # with outT matmuls lagging one pair behind the transposes ----
            outT = []
            a2t_state = {}

            def a2_transpose(pp):
                oc, sl = pp // 4, 128 * (pp % 4)
                pa = ps_big.tile([128, 512], BF16, tag="pbig")
                for t in range(4):
                    nc.tensor.transpose(
                        pa[:, 128 * t : 128 * (t + 1)],
                        A2n[t][oc][:, sl : sl + 128],
                        ident[:],
                    )
                pb = ps_small.tile([128, 128], BF16, tag="psmall")
                nc.tensor.transpose(
                    pb[:, 0:65], A2n[4][oc][0:65, sl : sl + 128], ident[0:65, 0:65]
                )
                a2t = sb.tile([128, 640], BF16, tag="a2t", bufs=4)
                nc.any.tensor_copy(a2t[:, 0:512], pa[:])
                nc.any.tensor_copy(a2t[:, 512:577], pb[:, 0:65])
                nc.any.memset(a2t[:, 577:640], 0.0)
                a2t_state[pp] = a2t

            def out_mm(pp):
                a2t = a2t_state.pop(pp)
                oa = ps_big.tile([128, 512], F32, tag="pbig")
                ob = ps_big.tile([128, 128], F32, tag="pbig")
                nc.tensor.matmul(
                    oa[:], QdBD[pp][:], a2t[:, 0:512], start=True, stop=True
                )
                nc.tensor.matmul(
                    ob[:], QdBD[pp][:], a2t[:, 512:640], start=True, stop=True
                )
                ot = sb.tile([128, 640], BF16, tag=f"ot{pp}", bufs=1)
                nc.any.tensor_copy(ot[:, 0:512], oa[:])
                nc.any.tensor_copy(ot[:, 512:640], ob[:])
                outT.append(ot)

            a2_transpose(0)
            a2_transpose(1)
            for pp in range(2, 8):
                a2_transpose(pp)
                out_mm(pp - 2)
            out_mm(6)
            out_mm(7)

            # ---- Phase 8: output projection + bias, then to DRAM ----
            build_bias()
            for t, (toff, rows) in enumerate(TOK):
                lo = toff if rows == 128 else 512
                for half in range(2):
                    cs = slice(512 * half, 512 * (half + 1))
                    ps = ps_big.tile([128, 512], F32, tag="pbig")
                    for cc in range(8):
                        nc.tensor.matmul(
                            ps[:],
                            outT[cc][:, lo : lo + 128],
                            wp[cc][:, cs],
                            start=(cc == 0),
                            stop=(cc == 7),
                        )
                    y = sb.tile([128, 512], F32, tag="y", bufs=3)
                    nc.vector.tensor_add(y[0:rows, :], ps[0:rows, :], bias[0:rows, cs])
                    nc.sync.dma_start(out_d[b, toff : toff + rows, cs], y[0:rows, :])

        for p in (ps_small, ps_big, sb, w_pool, const_pool):
            p.release()

    nc.compile()
    return nc


_NC_CACHE = {}


def _get_nc(nb: int = NB):
    if nb not in _NC_CACHE:
        _NC_CACHE[nb] = build_program(nb)
    return _NC_CACHE[nb]


def _split_fp8(a):
    """e4m3 value + e4m3 residual (round-to-nearest both times)."""
    hi = a.astype(ml_dtypes.float8_e4m3)
    lo = (a - hi.astype(np.float32)).astype(ml_dtypes.float8_e4m3)
    return hi, lo


def kernel(X, W_qkv, W_proj, b_proj, layer_idx=None):
    assert X.shape == (B, N, C)
    nc = _get_nc(NB)
    xt = np.zeros((B, C, 640), dtype=np.float32)
    xt[:, :, :N] = np.asarray(X, dtype=np.float32).transpose(0, 2, 1)
    xhi, xlo = _split_fp8(xt)
    wq32 = np.ascontiguousarray(np.asarray(W_qkv, dtype=np.float32).T) * 32.0
    whi, wlo = _split_fp8(wq32)
    wprojt = (
        np.ascontiguousarray(np.asarray(W_proj, dtype=np.float32).T) / 32.0
    ).astype(ml_dtypes.bfloat16)
    wbias = np.asarray(b_proj, dtype=np.float32).reshape(1, C).astype(
        ml_dtypes.bfloat16
    )
    in_maps = [
        {
            "xhi": xhi[NB * i : NB * (i + 1)],
            "xlo": xlo[NB * i : NB * (i + 1)],
            "whi": whi,
            "wlo": wlo,
            "wprojt": wprojt,
            "wbias": wbias,
        }
        for i in range(N_CORES)
    ]
    res = run_bass_kernel_spmd(nc, in_maps, core_ids=list(range(N_CORES)))
    out = np.concatenate([res.results[i]["out"] for i in range(N_CORES)], axis=0)
    return out.astype(np.float32)


if __name__ == "__main__":
    rng = np.random.default_rng(0)
    X = rng.standard_normal((B, N, C), dtype=np.float32)
    W_qkv = rng.standard_normal((3 * C, C), dtype=np.float32) * C**-0.5
    W_proj = rng.standard_normal((C, C), dtype=np.float32) * C**-0.5
    b_proj = np.zeros(C, dtype=np.float32)
    out = kernel(X, W_qkv, W_proj, b_proj, 1)
    print(out.shape, out.dtype)



# revision 15
# speedup vs baseline: 1.3065x; 1.1526x over previous
"""Trainium2 Bass kernel for two-stage pooled-query attention.

Problem (hardcoded):
    B=32, N=577, C=1024, H=16 heads, d=64, pooled queries 8x8 (3x3 mean over
    24x24 grid of non-cls tokens).
    qkv = X @ W_qkv.T ; pool Xq -> Qp ; s1 = softmax(Qp*s @ K^T) @ V ;
    s2 = softmax(Xq*s @ Qp^T) @ s1 ; out = s2 @ W_proj.T + b_proj

Strategy: pure data-parallel over batch across 8 NeuronCores (4 batches per
core, no collectives). The big QKV GEMM runs fp8 DoubleRow (K=256 per
instruction at 0.5 cycles/row) with a 3-term hi/lo hybrid split to hold
accuracy:
  X @ W ~= X_hi@W_hi + X_lo@W_hi + X_hi@W_lo, each operand split on the host
  into an e4m3 value plus an e4m3 residual (W_qkv pre-scaled by 32 so its
  residual clears the fp8 subnormal floor; the 32 cancels through the pooled
  query scale and W_proj/32).

The attention middle runs bf16 with no PE transposes:
  - Stage 1 is computed transposed, s1T [n, 2q] = qkT_k.T @ QpBD, so the
    A1 weights land directly in the lhsT layout Qd needs. The softmax
    denominator Z1 comes from a 1-column ones matmul (~free), and 1/Z1 is
    folded per-q-partition into stage 2's weights instead of into Qd.
  - Stage 2 is computed transposed too, E2T [2q, n] = exp(QpBD.T @ qkT_q),
    the per-(token,head) denominator is produced pre-broadcast by one
    block-diag-ones matmul, and a single fused scalar_tensor_tensor gives
    e2n = E2T * r1[q] * (1/Z2) -- both normalizations in one op. The output
    outT [c, n] = QdBD_raw @ e2n needs no A2 transpose either.

Batches are software-pipelined: batch b+1's QKV GEMM instructions are
emitted interleaved into batch b's attention middle, so the in-order PE
queue always has independent work while the middle waits on softmax.
"""

import os
import sys

import numpy as np

sys.path.insert(0, "/opt/trn_rl_repo")

import ml_dtypes  # noqa: E402

import concourse.tile as tile  # noqa: E402
from concourse import bacc, mybir  # noqa: E402
from concourse.bass_utils import run_bass_kernel_spmd  # noqa: E402

B, N, C = 32, 577, 1024
H, D = 16, 64
SCALE = D ** -0.5
N_CORES = 8
NB = B // N_CORES  # batches per core

BF16 = mybir.dt.bfloat16
F32 = mybir.dt.float32
FP8 = mybir.dt.float8e4
DR = mybir.MatmulPerfMode.DoubleRow
WSCALE = 32.0  # host pre-scale on W_qkv^T (keeps fp8 residuals normal)
MULT = mybir.AluOpType.mult

# token chunks of 577 = 4*128 + 65
TOK = [(0, 128), (128, 128), (256, 128), (384, 128), (512, 65)]
# free-dim chunks of 577 for wide matmuls: one full psum bank + a stub
NF = [(0, 512), (512, 65)]
EXP = mybir.ActivationFunctionType.Exp


def build_program(nb: int = NB):
    nc = bacc.Bacc("TRN2", target_bir_lowering=False, debug=False)

    xhi_d = nc.dram_tensor("xhi", [nb, C, 640], FP8, kind="ExternalInput")
    xlo_d = nc.dram_tensor("xlo", [nb, C, 640], FP8, kind="ExternalInput")
    whi_d = nc.dram_tensor("whi", [C, 3 * C], FP8, kind="ExternalInput")
    wlo_d = nc.dram_tensor("wlo", [C, 3 * C], FP8, kind="ExternalInput")
    wprojt_d = nc.dram_tensor("wprojt", [C, C], BF16, kind="ExternalInput")
    wbias_d = nc.dram_tensor("wbias", [1, C], BF16, kind="ExternalInput")
    out_d = nc.dram_tensor("out", [nb, N, C], BF16, kind="ExternalOutput")

    with tile.TileContext(nc) as tc:
        const_pool = tc.alloc_tile_pool(name="const", bufs=1)
        w_pool = tc.alloc_tile_pool(name="w", bufs=1)
        sb = tc.alloc_tile_pool(name="sb", bufs=2)
        ps_big = tc.alloc_tile_pool(name="ps_big", bufs=5, space="PSUM")
        ps_small = tc.alloc_tile_pool(name="ps_small", bufs=3, space="PSUM")

        ones = const_pool.tile([1, 128], BF16, tag="ones")
        nc.gpsimd.memset(ones[:], 1.0)
        ones128 = const_pool.tile([128, 1], BF16, tag="ones128")
        nc.gpsimd.memset(ones128[:], 1.0)
        # block-diag ones [2q, 128]: col j sums the q-rows of head(j)
        onesbd = const_pool.tile([128, 128], BF16, tag="onesbd")
        nc.gpsimd.memset(onesbd[:], 0.0)
        nc.gpsimd.memset(onesbd[0:64, 0:64], 1.0)
        nc.gpsimd.memset(onesbd[64:128, 64:128], 1.0)

        # first batch's X goes out before the (much larger) weight DMAs so
        # the QKV gemm can start immediately; weights stream behind. X and W
        # ship as fp8 hi/lo chunk-pair tiles [128, 2, cols] (dim 1 = the two
        # K=128 sub-chunks one DoubleRow matmul contracts).
        def load_x(b):
            tiles = []
            for c2 in range(4):
                src = slice(256 * c2, 256 * (c2 + 1))
                xh = sb.tile([128, 2, 640], FP8, tag=f"xh{c2}", bufs=2)
                nc.sync.dma_start(
                    xh[:], xhi_d[b, src, :].rearrange("(i p) n -> p i n", i=2)
                )
                xl = sb.tile([128, 2, 640], FP8, tag=f"xl{c2}", bufs=2)
                nc.sync.dma_start(
                    xl[:], xlo_d[b, src, :].rearrange("(i p) n -> p i n", i=2)
                )
                tiles.append((xh, xl))
            return tiles

        XT0 = load_x(0)

        wh, wl = [], []
        for c2 in range(4):
            wht = w_pool.tile([128, 2, 3 * C], FP8, tag=f"wh{c2}")
            wh.append(wht)
            wlt = w_pool.tile([128, 2, 3 * C], FP8, tag=f"wl{c2}")
            wl.append(wlt)
        for blk in range(6):
            cs = slice(512 * blk, 512 * (blk + 1))
            for c2 in range(4):
                src = slice(256 * c2, 256 * (c2 + 1))
                nc.sync.dma_start(
                    wh[c2][:, :, cs],
                    whi_d[src, cs].rearrange("(i p) n -> p i n", i=2),
                )
                nc.sync.dma_start(
                    wl[c2][:, :, cs],
                    wlo_d[src, cs].rearrange("(i p) n -> p i n", i=2),
                )
        wp = []
        for j in range(8):
            t = w_pool.tile([128, C], BF16, tag=f"wp{j}")
            nc.sync.dma_start(t[:], wprojt_d[128 * j : 128 * (j + 1), :])
            wp.append(t)
        wb = w_pool.tile([1, C], BF16, tag="wb")
        nc.sync.dma_start(wb[:], wbias_d[:])

        # persistent zeros: these tiles only ever get their "active" region
        # rewritten, so zero every rotation buffer once up front and never
        # memset in the loop.
        for _ in range(2):
            for j in range(8):
                qp = sb.tile([128, 128], BF16, tag=f"qp{j}")
                nc.gpsimd.memset(qp[:], 0.0)
            for pp in range(8):
                qd = sb.tile([128, 128], BF16, tag=f"qd{pp}")
                nc.gpsimd.memset(qd[:], 0.0)
        # (row 64 = token 576 is real and rewritten every batch; rows 65+ are
        # pad and must stay finite-zero. Partition ranges must start aligned,
        # so zero [64:128] once -- the loop re-writes row 64 before reading.)
        for _ in range(3):
            a1t = sb.tile([128, 5, 128], BF16, tag="a1t5", bufs=3)
            nc.gpsimd.memset(a1t[64:128, 4, :], 0.0)
        for _ in range(2):
            vt = sb.tile([128, C], BF16, tag="v4", bufs=2)
            nc.gpsimd.memset(vt[64:128, :], 0.0)

        # bias broadcast [128, 1024]; built lazily (first use is phase 8)
        bias = const_pool.tile([128, C], BF16, tag="bias")
        bias_built = [False]

        def build_bias():
            if bias_built[0]:
                return
            bias_built[0] = True
            for half in range(2):
                cs = slice(512 * half, 512 * (half + 1))
                bps = ps_big.tile([128, 512], F32, tag="pbig")
                nc.tensor.matmul(
                    bps[:], ones[0:1, :], wb[0:1, cs], start=True, stop=True
                )
                nc.any.tensor_copy(bias[:, cs], bps[:])

        # ---- front: QKV gemm + V + pooling for one batch, as thunks so the
        # emission can interleave into the previous batch's middle ----
        def front_thunks(b, XT):
            qkT = []
            V = []
            QpBD = []

            def phase2_chunk(cc):
                # q/k channels transposed: qkT [c, n] = 32 * true
                qt = sb.tile([128, 640], BF16, tag=f"qkt{cc}", bufs=2)
                csl = slice(128 * cc, 128 * (cc + 1))
                for ci, (n0, nw) in enumerate(NF):
                    ps = (ps_big if nw > 128 else ps_small).tile(
                        [128, nw], F32, tag="pbig" if nw > 128 else "psmall"
                    )
                    terms = (
                        [(wh[c2], XT[c2][0]) for c2 in range(4)]
                        + [(wl[c2], XT[c2][0]) for c2 in range(4)]
                        + [(wh[c2], XT[c2][1]) for c2 in range(4)]
                    )
                    for ti, (wt, xt_) in enumerate(terms):
                        nc.tensor.matmul(
                            ps[:],
                            wt[:, :, csl],
                            xt_[:, :, n0 : n0 + nw],
                            start=(ti == 0),
                            stop=(ti == 11),
                            perf_mode=DR,
                        )
                    if (cc + ci) % 2 == 0:
                        nc.vector.tensor_copy(qt[:, n0 : n0 + nw], ps[:])
                    else:
                        nc.scalar.copy(qt[:, n0 : n0 + nw], ps[:])
                qkT.append(qt)

            def phase3_tile(t):
                # V natural layout [n, c] = 32 * true; pad rows stay zero
                # because X pad columns are zero in both hi and lo.
                toff, rows = TOK[t]
                vt = sb.tile([128, C], BF16, tag=f"v{t}", bufs=2)
                lo = toff if rows == 128 else 512
                lw = 2 * rows
                for h2 in range(2):
                    ps = ps_big.tile([128, 512], F32, tag="pbig")
                    cs = slice(2048 + 512 * h2, 2048 + 512 * (h2 + 1))
                    terms = (
                        [(XT[c2][0], wh[c2]) for c2 in range(4)]
                        + [(XT[c2][0], wl[c2]) for c2 in range(4)]
                        + [(XT[c2][1], wh[c2]) for c2 in range(4)]
                    )
                    for ti, (xt_, wt) in enumerate(terms):
                        nc.tensor.matmul(
                            ps[0:rows, :],
                            xt_[:, :, lo : lo + rows],
                            wt[:, :, cs],
                            start=(ti == 0),
                            stop=(ti == 11),
                            perf_mode=DR,
                        )
                    nc.scalar.copy(vt[0:rows, 512 * h2 : 512 * (h2 + 1)], ps[0:rows, :])
                V.append(vt)

            def pool_j(j):
                # pooled queries, block-diag [c, 2q] per pair; qkT carries
                # 32x, so alpha makes qp = SCALE * Qp_true / 32 which renders
                # both score matmuls exact.
                qsum = sb.tile([128, 64], F32, tag="qsum", bufs=3)
                view = qkT[j][:, 0:576].rearrange(
                    "p (pr dr pc dc) -> p pr pc dr dc", pr=8, dr=3, pc=8, dc=3
                )
                nc.vector.reduce_sum(qsum[:], view, axis=mybir.AxisListType.XY)
                alpha = SCALE / (9.0 * WSCALE * WSCALE)
                qp = sb.tile([128, 128], BF16, tag=f"qp{j}")
                nc.gpsimd.tensor_scalar_mul(qp[0:64, 0:64], qsum[0:64, :], alpha)
                nc.gpsimd.tensor_scalar_mul(qp[64:128, 64:128], qsum[64:128, :], alpha)
                QpBD.append(qp)

            thunks = [lambda cc=cc: phase2_chunk(cc) for cc in range(16)]
            thunks += [lambda t=t: phase3_tile(t) for t in range(5)]
            thunks += [lambda j=j: pool_j(j) for j in range(8)]
            return thunks, (qkT, V, QpBD)

        # ---- back: attention middle + output projection for one batch ----
        def back_emit(b, state, filler):
            qkT, V, QpBD = state
            fill_i = [0]

            def fill(k):
                for _ in range(k):
                    if fill_i[0] < len(filler):
                        filler[fill_i[0]]()
                        fill_i[0] += 1

            A1T = [None] * 8
            E2 = [None] * 8
            RZ = [None] * 8
            R1 = [None] * 8
            E2N = [None] * 8
            QdBD = [None] * 8
            outT = [None] * 8

            def scores(pp):
                # s1T [n, 2q]: lhsT = qkT_k chunk, rhs = QpBD; exp rows
                # limited to real tokens (pad rows stay prologue-zero)
                pa = ps_big.tile([128, 512], F32, tag="pbig")
                for t in range(4):
                    nc.tensor.matmul(
                        pa[:, 128 * t : 128 * (t + 1)],
                        qkT[8 + pp][:, 128 * t : 128 * (t + 1)],
                        QpBD[pp][:],
                        start=True,
                        stop=True,
                    )
                pb = ps_small.tile([128, 128], F32, tag="psmall")
                nc.tensor.matmul(
                    pb[:], qkT[8 + pp][:, 512:640], QpBD[pp][:], start=True, stop=True
                )
                a1t = sb.tile([128, 5, 128], BF16, tag="a1t5", bufs=3)
                nc.scalar.activation(
                    a1t[:, 0:4, :], pa[:].rearrange("p (t q) -> p t q", t=4), EXP
                )
                nc.scalar.activation(a1t[0:65, 4, :], pb[0:65, :], EXP)
                A1T[pp] = a1t

                # s2T [2q, n] = QpBD.T @ qkT_q; exp straight to E2 bf16
                e2 = sb.tile([128, 640], BF16, tag="e2", bufs=3)
                for n0, nw in NF:
                    ps = (ps_big if nw > 128 else ps_small).tile(
                        [128, nw], F32, tag="pbig" if nw > 128 else "psmall"
                    )
                    nc.tensor.matmul(
                        ps[:], QpBD[pp][:], qkT[pp][:, n0 : n0 + nw],
                        start=True, stop=True,
                    )
                    nc.scalar.activation(e2[:, n0 : n0 + nw], ps[:], EXP)
                E2[pp] = e2

            def tails(pp):
                a1t = A1T[pp]
                # Qd_raw pair product [2q, dv] plus a Z1 ones-column, both
                # accumulated into one psum bank; diag blocks kept raw
                # (1/Z1 is folded into e2n below)
                qd_ps = ps_small.tile([128, 129], F32, tag="psmall")
                for t in range(5):
                    nc.tensor.matmul(
                        qd_ps[:, 0:128],
                        a1t[:, t, :],
                        V[t][:, 128 * pp : 128 * (pp + 1)],
                        start=(t == 0),
                        stop=(t == 4),
                    )
                for t in range(5):
                    nc.tensor.matmul(
                        qd_ps[:, 128:129],
                        a1t[:, t, :],
                        ones128[:],
                        start=(t == 0),
                        stop=(t == 4),
                    )
                r1 = sb.tile([128, 1], F32, tag="r1", bufs=3)
                nc.vector.reciprocal(r1[:], qd_ps[:, 128:129])
                R1[pp] = r1
                qd = sb.tile([128, 128], BF16, tag=f"qd{pp}")
                nc.vector.tensor_copy(qd[0:64, 0:64], qd_ps[0:64, 0:64])
                nc.vector.tensor_copy(qd[64:128, 64:128], qd_ps[64:128, 64:128])
                QdBD[pp] = qd

                # Z2 pre-broadcast over q-rows, then 1/Z2
                e2 = E2[pp]
                rz = sb.tile([128, 640], F32, tag="rz", bufs=3)
                for n0, nw in NF:
                    zb = (ps_big if nw > 128 else ps_small).tile(
                        [128, nw], F32, tag="pbig" if nw > 128 else "psmall"
                    )
                    nc.tensor.matmul(
                        zb[:], onesbd[:], e2[:, n0 : n0 + nw], start=True, stop=True
                    )
                    nc.vector.reciprocal(rz[:, n0 : n0 + nw], zb[:])
                RZ[pp] = rz

                # fused: e2n = (E2 * r1[q-partition]) * (1/Z2)
                e2n = sb.tile([128, 640], BF16, tag="e2n", bufs=3)
                nc.vector.scalar_tensor_tensor(
                    e2n[:, 0:577], e2[:, 0:577], r1[:, 0:1], rz[:, 0:577],
                    op0=MULT, op1=MULT,
                )
                E2N[pp] = e2n

            def out_mm(pp):
                e2n = E2N[pp]
                oa = ps_big.tile([128, 512], F32, tag="pbig")
                ob = ps_small.tile([128, 65], F32, tag="psmall")
                nc.tensor.matmul(
                    oa[:], QdBD[pp][:], e2n[:, 0:512], start=True, stop=True
                )
                nc.tensor.matmul(
                    ob[:], QdBD[pp][:], e2n[:, 512:577], start=True, stop=True
                )
                ot = sb.tile([128, 640], BF16, tag=f"ot{pp}", bufs=2)
                if pp % 2 == 0:
                    nc.scalar.copy(ot[:, 0:512], oa[:])
                    nc.scalar.copy(ot[:, 512:577], ob[:])
                else:
                    nc.vector.tensor_copy(ot[:, 0:512], oa[:])
                    nc.vector.tensor_copy(ot[:, 512:577], ob[:])
                outT[pp] = ot

            for pp in range(8):
                scores(pp)
                if pp >= 1:
                    tails(pp - 1)
                if pp >= 2:
                    out_mm(pp - 2)
                fill(2)
            tails(7)
            out_mm(6)
            fill(2)
            out_mm(7)

            # ---- output projection + bias -> bf16 -> DRAM ----
            build_bias()
            for t, (toff, rows) in enumerate(TOK):
                lo = toff if rows == 128 else 512
                for half in range(2):
                    cs = slice(512 * half, 512 * (half + 1))
                    ps = ps_big.tile([128, 512], F32, tag="pbig")
                    for cc in range(8):
                        nc.tensor.matmul(
                            ps[0:rows, :],
                            outT[cc][:, lo : lo + rows],
                            wp[cc][:, cs],
                            start=(cc == 0),
                            stop=(cc == 7),
                        )
                    y = sb.tile([128, 512], BF16, tag="y", bufs=3)
                    if (t + half) % 2 == 0:
                        nc.vector.tensor_tensor(
                            y[0:rows, :], ps[0:rows, :], bias[0:rows, cs],
                            op=mybir.AluOpType.add,
                        )
                    else:
                        nc.scalar.activation(
                            y[0:rows, :], ps[0:rows, :],
                            mybir.ActivationFunctionType.Copy,
                        )
                        nc.vector.tensor_tensor(
                            y[0:rows, :], y[0:rows, :], bias[0:rows, cs],
                            op=mybir.AluOpType.add,
                        )
                    nc.sync.dma_start(out_d[b, toff : toff + rows, cs], y[0:rows, :])
                    fill(1)
            fill(100)

        repeat = int(os.environ.get("KERNEL_REPEAT", "1"))
        total = nb * repeat
        # emit batch 0's front directly; every later front interleaves into
        # the previous batch's middle as PE filler.
        thunks, state = front_thunks(0, XT0)
        for th in thunks:
            th()
        for i in range(total):
            b = i % nb
            if i + 1 < total:
                XTn = load_x((i + 1) % nb)
                nxt_thunks, nxt_state = front_thunks((i + 1) % nb, XTn)
            else:
                nxt_thunks, nxt_state = [], None
            back_emit(b, state, nxt_thunks)
            state = nxt_state

        for p in (ps_small, ps_big, sb, w_pool, const_pool):
            p.release()

    nc.compile()
    return nc


_NC_CACHE = {}


def _get_nc(nb: int = NB):
    if nb not in _NC_CACHE:
        _NC_CACHE[nb] = build_program(nb)
    return _NC_CACHE[nb]


def _split_fp8(a):
    """e4m3 value + e4m3 residual (round-to-nearest both times)."""
    hi = a.astype(ml_dtypes.float8_e4m3)
    lo = (a - hi.astype(np.float32)).astype(ml_dtypes.float8_e4m3)
    return hi, lo


def kernel(X, W_qkv, W_proj, b_proj, layer_idx=None):
    assert X.shape == (B, N, C)
    nc = _get_nc(NB)
    xt = np.zeros((B, C, 640), dtype=np.float32)
    xt[:, :, :N] = np.asarray(X, dtype=np.float32).transpose(0, 2, 1)
    xhi, xlo = _split_fp8(xt)
    wq32 = np.ascontiguousarray(np.asarray(W_qkv, dtype=np.float32).T) * 32.0
    whi, wlo = _split_fp8(wq32)
    wprojt = (
        np.ascontiguousarray(np.asarray(W_proj, dtype=np.float32).T) / 32.0
    ).astype(ml_dtypes.bfloat16)
    wbias = np.asarray(b_proj, dtype=np.float32).reshape(1, C).astype(
        ml_dtypes.bfloat16
    )
    in_maps = [
        {
            "xhi": xhi[NB * i : NB * (i + 1)],
            "xlo": xlo[NB * i : NB * (i + 1)],
            "whi": whi,
            "wlo": wlo,
            "wprojt": wprojt,
            "wbias": wbias,
        }
        for i in range(N_CORES)
    ]
    res = run_bass_kernel_spmd(nc, in_maps, core_ids=list(range(N_CORES)))
    out = np.concatenate([res.results[i]["out"] for i in range(N_CORES)], axis=0)
    return out.astype(np.float32)


if __name__ == "__main__":
    rng = np.random.default_rng(0)
    X = rng.standard_normal((B, N, C), dtype=np.float32)
    W_qkv = rng.standard_normal((3 * C, C), dtype=np.float32) * C**-0.5
    W_proj = rng.standard_normal((C, C), dtype=np.float32) * C**-0.5
    b_proj = np.zeros(C, dtype=np.float32)
    out = kernel(X, W_qkv, W_proj, b_proj, 1)
    print(out.shape, out.dtype)


# revision 16
# speedup vs baseline: 1.3686x; 1.0476x over previous
"""Trainium2 Bass kernel for two-stage pooled-query attention.

Problem (hardcoded):
    B=32, N=577, C=1024, H=16 heads, d=64, pooled queries 8x8 (3x3 mean over
    24x24 grid of non-cls tokens).
    qkv = X @ W_qkv.T ; pool Xq -> Qp ; s1 = softmax(Qp*s @ K^T) @ V ;
    s2 = softmax(Xq*s @ Qp^T) @ s1 ; out = s2 @ W_proj.T + b_proj

Strategy: pure data-parallel over batch across 8 NeuronCores (4 batches per
core, no collectives). The big QKV GEMM runs fp8 DoubleRow (K=256 per
instruction at 0.5 cycles/row) with a 3-term hi/lo hybrid split to hold
accuracy:
  X @ W ~= X_hi@W_hi + X_lo@W_hi + X_hi@W_lo, each operand split on the host
  into an e4m3 value plus an e4m3 residual (W_qkv pre-scaled by 32 so its
  residual clears the fp8 subnormal floor; the 32 cancels through the pooled
  query scale and W_proj/32).

The attention middle runs bf16 with no PE transposes:
  - Stage 1 is computed transposed, s1T [n, 2q] = qkT_k.T @ QpBD, so the
    A1 weights land directly in the lhsT layout Qd needs. The softmax
    denominator Z1 comes from a 1-column ones matmul (~free), and 1/Z1 is
    folded per-q-partition into stage 2's weights instead of into Qd.
  - Stage 2 is computed transposed too, E2T [2q, n] = exp(QpBD.T @ qkT_q),
    the per-(token,head) denominator is produced pre-broadcast by one
    block-diag-ones matmul, and a single fused scalar_tensor_tensor gives
    e2n = E2T * r1[q] * (1/Z2) -- both normalizations in one op. The output
    outT [c, n] = QdBD_raw @ e2n needs no A2 transpose either.

Batches are software-pipelined: batch b+1's QKV GEMM instructions are
emitted interleaved into batch b's attention middle, so the in-order PE
queue always has independent work while the middle waits on softmax.
"""

import os
import sys

import numpy as np

sys.path.insert(0, "/opt/trn_rl_repo")

import ml_dtypes  # noqa: E402

import concourse.tile as tile  # noqa: E402
from concourse import bacc, mybir  # noqa: E402
from concourse.bass_utils import run_bass_kernel_spmd  # noqa: E402

B, N, C = 32, 577, 1024
H, D = 16, 64
SCALE = D ** -0.5
N_CORES = 8
NB = B // N_CORES  # batches per core

BF16 = mybir.dt.bfloat16
F32 = mybir.dt.float32
FP8 = mybir.dt.float8e4
DR = mybir.MatmulPerfMode.DoubleRow
WSCALE = 32.0  # host pre-scale on W_qkv^T (keeps fp8 residuals normal)
MULT = mybir.AluOpType.mult

# token chunks of 577 = 4*128 + 65
TOK = [(0, 128), (128, 128), (256, 128), (384, 128), (512, 65)]
# free-dim chunks of 577 for wide matmuls: one full psum bank + a stub
NF = [(0, 512), (512, 65)]
EXP = mybir.ActivationFunctionType.Exp


def build_program(nb: int = NB):
    nc = bacc.Bacc("TRN2", target_bir_lowering=False, debug=False)

    xhi_d = nc.dram_tensor("xhi", [nb, C, 640], FP8, kind="ExternalInput")
    xlo_d = nc.dram_tensor("xlo", [nb, C, 640], FP8, kind="ExternalInput")
    whi_d = nc.dram_tensor("whi", [C, 3 * C], FP8, kind="ExternalInput")
    wlo_d = nc.dram_tensor("wlo", [C, 3 * C], FP8, kind="ExternalInput")
    wphi_d = nc.dram_tensor("wphi", [C, C], FP8, kind="ExternalInput")
    wplo_d = nc.dram_tensor("wplo", [C, C], FP8, kind="ExternalInput")
    wbias_d = nc.dram_tensor("wbias", [1, C], BF16, kind="ExternalInput")
    out_d = nc.dram_tensor("out", [nb, N, C], BF16, kind="ExternalOutput")

    with tile.TileContext(nc) as tc:
        const_pool = tc.alloc_tile_pool(name="const", bufs=1)
        w_pool = tc.alloc_tile_pool(name="w", bufs=1)
        sb = tc.alloc_tile_pool(name="sb", bufs=2)
        ps_big = tc.alloc_tile_pool(name="ps_big", bufs=5, space="PSUM")
        ps_small = tc.alloc_tile_pool(name="ps_small", bufs=3, space="PSUM")

        ones = const_pool.tile([1, 128], BF16, tag="ones")
        nc.gpsimd.memset(ones[:], 1.0)
        ones128 = const_pool.tile([128, 1], BF16, tag="ones128")
        nc.gpsimd.memset(ones128[:], 1.0)
        # block-diag ones [2q, 128]: col j sums the q-rows of head(j)
        onesbd = const_pool.tile([128, 128], BF16, tag="onesbd")
        nc.gpsimd.memset(onesbd[:], 0.0)
        nc.gpsimd.memset(onesbd[0:64, 0:64], 1.0)
        nc.gpsimd.memset(onesbd[64:128, 64:128], 1.0)

        # first batch's X goes out before the (much larger) weight DMAs so
        # the QKV gemm can start immediately; weights stream behind. X and W
        # ship as fp8 hi/lo chunk-pair tiles [128, 2, cols] (dim 1 = the two
        # K=128 sub-chunks one DoubleRow matmul contracts).
        def load_x(b):
            tiles = []
            for c2 in range(4):
                src = slice(256 * c2, 256 * (c2 + 1))
                xh = sb.tile([128, 2, 640], FP8, tag=f"xh{c2}", bufs=2)
                nc.sync.dma_start(
                    xh[:], xhi_d[b, src, :].rearrange("(i p) n -> p i n", i=2)
                )
                xl = sb.tile([128, 2, 640], FP8, tag=f"xl{c2}", bufs=2)
                nc.sync.dma_start(
                    xl[:], xlo_d[b, src, :].rearrange("(i p) n -> p i n", i=2)
                )
                tiles.append((xh, xl))
            return tiles

        XT0 = load_x(0)

        wh, wl = [], []
        for c2 in range(4):
            wht = w_pool.tile([128, 2, 3 * C], FP8, tag=f"wh{c2}")
            wh.append(wht)
            wlt = w_pool.tile([128, 2, 3 * C], FP8, tag=f"wl{c2}")
            wl.append(wlt)
        for blk in range(6):
            cs = slice(512 * blk, 512 * (blk + 1))
            for c2 in range(4):
                src = slice(256 * c2, 256 * (c2 + 1))
                nc.sync.dma_start(
                    wh[c2][:, :, cs],
                    whi_d[src, cs].rearrange("(i p) n -> p i n", i=2),
                )
                nc.sync.dma_start(
                    wl[c2][:, :, cs],
                    wlo_d[src, cs].rearrange("(i p) n -> p i n", i=2),
                )
        wph, wpl = [], []
        for c2 in range(4):
            src_ = slice(256 * c2, 256 * (c2 + 1))
            t = w_pool.tile([128, 2, C], FP8, tag=f"wph{c2}")
            nc.sync.dma_start(t[:], wphi_d[src_, :].rearrange("(i p) n -> p i n", i=2))
            wph.append(t)
            t = w_pool.tile([128, 2, C], FP8, tag=f"wpl{c2}")
            nc.sync.dma_start(t[:], wplo_d[src_, :].rearrange("(i p) n -> p i n", i=2))
            wpl.append(t)
        wb = w_pool.tile([1, C], BF16, tag="wb")
        nc.sync.dma_start(wb[:], wbias_d[:])

        # persistent zeros: these tiles only ever get their "active" region
        # rewritten, so zero every rotation buffer once up front and never
        # memset in the loop.
        for _ in range(2):
            for j in range(8):
                qp = sb.tile([128, 128], BF16, tag=f"qp{j}")
                nc.gpsimd.memset(qp[:], 0.0)
            for pp in range(8):
                qd = sb.tile([128, 128], BF16, tag=f"qd{pp}")
                nc.gpsimd.memset(qd[:], 0.0)
        # (row 64 = token 576 is real and rewritten every batch; rows 65+ are
        # pad and must stay finite-zero. Partition ranges must start aligned,
        # so zero [64:128] once -- the loop re-writes row 64 before reading.)
        for _ in range(3):
            a1t = sb.tile([128, 5, 128], BF16, tag="a1t5", bufs=3)
            nc.gpsimd.memset(a1t[64:128, 4, :], 0.0)
        for _ in range(2):
            vt = sb.tile([128, C], BF16, tag="v4", bufs=2)
            nc.gpsimd.memset(vt[64:128, :], 0.0)

        # bias broadcast [128, 1024]; built lazily (first use is phase 8)
        bias = const_pool.tile([128, C], BF16, tag="bias")
        bias_built = [False]

        def build_bias():
            if bias_built[0]:
                return
            bias_built[0] = True
            for half in range(2):
                cs = slice(512 * half, 512 * (half + 1))
                bps = ps_big.tile([128, 512], F32, tag="pbig")
                nc.tensor.matmul(
                    bps[:], ones[0:1, :], wb[0:1, cs], start=True, stop=True
                )
                nc.any.tensor_copy(bias[:, cs], bps[:])

        # ---- front: QKV gemm + V + pooling for one batch, as thunks so the
        # emission can interleave into the previous batch's middle ----
        def front_thunks(b, XT):
            qkT = []
            V = []
            QpBD = []

            def phase2_chunk(cc):
                # q/k channels transposed: qkT [c, n] = 32 * true
                qt = sb.tile([128, 640], BF16, tag=f"qkt{cc}", bufs=2)
                csl = slice(128 * cc, 128 * (cc + 1))
                for ci, (n0, nw) in enumerate(NF):
                    ps = (ps_big if nw > 128 else ps_small).tile(
                        [128, nw], F32, tag="pbig" if nw > 128 else "psmall"
                    )
                    terms = (
                        [(wh[c2], XT[c2][0]) for c2 in range(4)]
                        + [(wl[c2], XT[c2][0]) for c2 in range(4)]
                        + [(wh[c2], XT[c2][1]) for c2 in range(4)]
                    )
                    for ti, (wt, xt_) in enumerate(terms):
                        nc.tensor.matmul(
                            ps[:],
                            wt[:, :, csl],
                            xt_[:, :, n0 : n0 + nw],
                            start=(ti == 0),
                            stop=(ti == 11),
                            perf_mode=DR,
                        )
                    if (cc + ci) % 2 == 0:
                        nc.vector.tensor_copy(qt[:, n0 : n0 + nw], ps[:])
                    else:
                        nc.scalar.copy(qt[:, n0 : n0 + nw], ps[:])
                qkT.append(qt)

            def phase3_tile(t):
                # V natural layout [n, c] = 32 * true; pad rows stay zero
                # because X pad columns are zero in both hi and lo.
                toff, rows = TOK[t]
                vt = sb.tile([128, C], BF16, tag=f"v{t}", bufs=2)
                lo = toff if rows == 128 else 512
                lw = 2 * rows
                for h2 in range(2):
                    ps = ps_big.tile([128, 512], F32, tag="pbig")
                    cs = slice(2048 + 512 * h2, 2048 + 512 * (h2 + 1))
                    terms = (
                        [(XT[c2][0], wh[c2]) for c2 in range(4)]
                        + [(XT[c2][0], wl[c2]) for c2 in range(4)]
                        + [(XT[c2][1], wh[c2]) for c2 in range(4)]
                    )
                    for ti, (xt_, wt) in enumerate(terms):
                        nc.tensor.matmul(
                            ps[0:rows, :],
                            xt_[:, :, lo : lo + rows],
                            wt[:, :, cs],
                            start=(ti == 0),
                            stop=(ti == 11),
                            perf_mode=DR,
                        )
                    nc.scalar.copy(vt[0:rows, 512 * h2 : 512 * (h2 + 1)], ps[0:rows, :])
                V.append(vt)

            def pool_j(j):
                # pooled queries, block-diag [c, 2q] per pair; qkT carries
                # 32x, so alpha makes qp = SCALE * Qp_true / 32 which renders
                # both score matmuls exact.
                qsum = sb.tile([128, 64], F32, tag="qsum", bufs=3)
                view = qkT[j][:, 0:576].rearrange(
                    "p (pr dr pc dc) -> p pr pc dr dc", pr=8, dr=3, pc=8, dc=3
                )
                nc.vector.reduce_sum(qsum[:], view, axis=mybir.AxisListType.XY)
                alpha = SCALE / (9.0 * WSCALE * WSCALE)
                qp = sb.tile([128, 128], BF16, tag=f"qp{j}")
                nc.gpsimd.tensor_scalar_mul(qp[0:64, 0:64], qsum[0:64, :], alpha)
                nc.gpsimd.tensor_scalar_mul(qp[64:128, 64:128], qsum[64:128, :], alpha)
                QpBD.append(qp)

            thunks = [lambda cc=cc: phase2_chunk(cc) for cc in range(16)]
            thunks += [lambda t=t: phase3_tile(t) for t in range(5)]
            thunks += [lambda j=j: pool_j(j) for j in range(8)]
            return thunks, (qkT, V, QpBD)

        # ---- back: attention middle + output projection for one batch ----
        def back_emit(b, state, filler):
            qkT, V, QpBD = state
            fill_i = [0]

            def fill(k):
                for _ in range(k):
                    if fill_i[0] < len(filler):
                        filler[fill_i[0]]()
                        fill_i[0] += 1

            A1T = [None] * 8
            E2 = [None] * 8
            RZ = [None] * 8
            R1 = [None] * 8
            E2N = [None] * 8
            QdBD = [None] * 8
            OTH = [None] * 4
            OTL = [None] * 4

            def scores(pp):
                # s1T [n, 2q]: lhsT = qkT_k chunk, rhs = QpBD; exp rows
                # limited to real tokens (pad rows stay prologue-zero)
                pa = ps_big.tile([128, 512], F32, tag="pbig")
                for t in range(4):
                    nc.tensor.matmul(
                        pa[:, 128 * t : 128 * (t + 1)],
                        qkT[8 + pp][:, 128 * t : 128 * (t + 1)],
                        QpBD[pp][:],
                        start=True,
                        stop=True,
                    )
                pb = ps_small.tile([128, 128], F32, tag="psmall")
                nc.tensor.matmul(
                    pb[:], qkT[8 + pp][:, 512:640], QpBD[pp][:], start=True, stop=True
                )
                a1t = sb.tile([128, 5, 128], BF16, tag="a1t5", bufs=3)
                nc.scalar.activation(
                    a1t[:, 0:4, :], pa[:].rearrange("p (t q) -> p t q", t=4), EXP
                )
                nc.scalar.activation(a1t[0:65, 4, :], pb[0:65, :], EXP)
                A1T[pp] = a1t

                # s2T [2q, n] = QpBD.T @ qkT_q; exp straight to E2 bf16
                e2 = sb.tile([128, 640], BF16, tag="e2", bufs=3)
                for n0, nw in NF:
                    ps = (ps_big if nw > 128 else ps_small).tile(
                        [128, nw], F32, tag="pbig" if nw > 128 else "psmall"
                    )
                    nc.tensor.matmul(
                        ps[:], QpBD[pp][:], qkT[pp][:, n0 : n0 + nw],
                        start=True, stop=True,
                    )
                    nc.scalar.activation(e2[:, n0 : n0 + nw], ps[:], EXP)
                E2[pp] = e2

            def tails(pp):
                a1t = A1T[pp]
                # Qd_raw pair product [2q, dv] plus a Z1 ones-column, both
                # accumulated into one psum bank; diag blocks kept raw
                # (1/Z1 is folded into e2n below)
                qd_ps = ps_small.tile([128, 129], F32, tag="psmall")
                for t in range(5):
                    nc.tensor.matmul(
                        qd_ps[:, 0:128],
                        a1t[:, t, :],
                        V[t][:, 128 * pp : 128 * (pp + 1)],
                        start=(t == 0),
                        stop=(t == 4),
                    )
                for t in range(5):
                    nc.tensor.matmul(
                        qd_ps[:, 128:129],
                        a1t[:, t, :],
                        ones128[:],
                        start=(t == 0),
                        stop=(t == 4),
                    )
                r1 = sb.tile([128, 1], F32, tag="r1", bufs=3)
                nc.vector.reciprocal(r1[:], qd_ps[:, 128:129])
                R1[pp] = r1
                qd = sb.tile([128, 128], BF16, tag=f"qd{pp}")
                nc.vector.tensor_copy(qd[0:64, 0:64], qd_ps[0:64, 0:64])
                nc.vector.tensor_copy(qd[64:128, 64:128], qd_ps[64:128, 64:128])
                QdBD[pp] = qd

                # Z2 pre-broadcast over q-rows, then 1/Z2
                e2 = E2[pp]
                rz = sb.tile([128, 640], F32, tag="rz", bufs=3)
                for n0, nw in NF:
                    zb = (ps_big if nw > 128 else ps_small).tile(
                        [128, nw], F32, tag="pbig" if nw > 128 else "psmall"
                    )
                    nc.tensor.matmul(
                        zb[:], onesbd[:], e2[:, n0 : n0 + nw], start=True, stop=True
                    )
                    nc.vector.reciprocal(rz[:, n0 : n0 + nw], zb[:])
                RZ[pp] = rz

                # fused: e2n = (E2 * r1[q-partition]) * (1/Z2)
                e2n = sb.tile([128, 640], BF16, tag="e2n", bufs=3)
                nc.vector.scalar_tensor_tensor(
                    e2n[:, 0:577], e2[:, 0:577], r1[:, 0:1], rz[:, 0:577],
                    op0=MULT, op1=MULT,
                )
                E2N[pp] = e2n

            def out_mm(pp):
                e2n = E2N[pp]
                oa = ps_big.tile([128, 512], F32, tag="pbig")
                ob = ps_small.tile([128, 65], F32, tag="psmall")
                nc.tensor.matmul(
                    oa[:], QdBD[pp][:], e2n[:, 0:512], start=True, stop=True
                )
                nc.tensor.matmul(
                    ob[:], QdBD[pp][:], e2n[:, 512:577], start=True, stop=True
                )
                c2, sub = pp // 2, pp % 2
                if sub == 0:
                    oth = sb.tile([128, 2, 640], FP8, tag=f"oth{c2}", bufs=2)
                    otl = sb.tile([128, 2, 640], FP8, tag=f"otl{c2}", bufs=2)
                    OTH[c2], OTL[c2] = oth, otl
                oth, otl = OTH[c2], OTL[c2]
                # hi = fp8(outT); lo = fp8(outT - hi), split across engines
                SUB = mybir.AluOpType.subtract
                nc.scalar.copy(oth[:, sub, 0:512], oa[:])
                nc.scalar.copy(oth[:, sub, 512:577], ob[:])
                nc.vector.tensor_tensor(
                    otl[:, sub, 0:512], oa[:], oth[:, sub, 0:512], op=SUB
                )
                nc.vector.tensor_tensor(
                    otl[:, sub, 512:577], ob[:], oth[:, sub, 512:577], op=SUB
                )

            for pp in range(8):
                scores(pp)
                if pp >= 1:
                    tails(pp - 1)
                if pp >= 2:
                    out_mm(pp - 2)
                fill(2)
            tails(7)
            out_mm(6)
            fill(2)
            out_mm(7)

            # ---- output projection + bias -> bf16 -> DRAM ----
            build_bias()
            for t, (toff, rows) in enumerate(TOK):
                lo = toff if rows == 128 else 512
                for half in range(2):
                    cs = slice(512 * half, 512 * (half + 1))
                    ps = ps_big.tile([128, 512], F32, tag="pbig")
                    terms = (
                        [(OTH[c2], wph[c2]) for c2 in range(4)]
                        + [(OTL[c2], wph[c2]) for c2 in range(4)]
                        + [(OTH[c2], wpl[c2]) for c2 in range(4)]
                    )
                    for ti, (ot_, wt) in enumerate(terms):
                        nc.tensor.matmul(
                            ps[0:rows, :],
                            ot_[:, :, lo : lo + rows],
                            wt[:, :, cs],
                            start=(ti == 0),
                            stop=(ti == 11),
                            perf_mode=DR,
                        )
                    y = sb.tile([128, 512], BF16, tag="y", bufs=3)
                    if (t + half) % 2 == 0:
                        nc.vector.tensor_tensor(
                            y[0:rows, :], ps[0:rows, :], bias[0:rows, cs],
                            op=mybir.AluOpType.add,
                        )
                    else:
                        nc.scalar.activation(
                            y[0:rows, :], ps[0:rows, :],
                            mybir.ActivationFunctionType.Copy,
                        )
                        nc.vector.tensor_tensor(
                            y[0:rows, :], y[0:rows, :], bias[0:rows, cs],
                            op=mybir.AluOpType.add,
                        )
                    nc.sync.dma_start(out_d[b, toff : toff + rows, cs], y[0:rows, :])
                    fill(1)
            fill(100)

        repeat = int(os.environ.get("KERNEL_REPEAT", "1"))
        total = nb * repeat
        # emit batch 0's front directly; every later front interleaves into
        # the previous batch's middle as PE filler.
        thunks, state = front_thunks(0, XT0)
        for th in thunks:
            th()
        for i in range(total):
            b = i % nb
            if i + 1 < total:
                XTn = load_x((i + 1) % nb)
                nxt_thunks, nxt_state = front_thunks((i + 1) % nb, XTn)
            else:
                nxt_thunks, nxt_state = [], None
            back_emit(b, state, nxt_thunks)
            state = nxt_state

        for p in (ps_small, ps_big, sb, w_pool, const_pool):
            p.release()

    nc.compile()
    return nc


_NC_CACHE = {}


def _get_nc(nb: int = NB):
    if nb not in _NC_CACHE:
        _NC_CACHE[nb] = build_program(nb)
    return _NC_CACHE[nb]


def _split_fp8(a):
    """e4m3 value + e4m3 residual (round-to-nearest both times)."""
    hi = a.astype(ml_dtypes.float8_e4m3)
    lo = (a - hi.astype(np.float32)).astype(ml_dtypes.float8_e4m3)
    return hi, lo


def kernel(X, W_qkv, W_proj, b_proj, layer_idx=None):
    assert X.shape == (B, N, C)
    nc = _get_nc(NB)
    xt = np.zeros((B, C, 640), dtype=np.float32)
    xt[:, :, :N] = np.asarray(X, dtype=np.float32).transpose(0, 2, 1)
    xhi, xlo = _split_fp8(xt)
    wq32 = np.ascontiguousarray(np.asarray(W_qkv, dtype=np.float32).T) * 32.0
    whi, wlo = _split_fp8(wq32)
    wp32 = np.ascontiguousarray(np.asarray(W_proj, dtype=np.float32).T) * 32.0
    wphi, wplo = _split_fp8(wp32)
    wbias = (np.asarray(b_proj, dtype=np.float32).reshape(1, C) * 1024.0).astype(
        ml_dtypes.bfloat16
    )
    in_maps = [
        {
            "xhi": xhi[NB * i : NB * (i + 1)],
            "xlo": xlo[NB * i : NB * (i + 1)],
            "whi": whi,
            "wlo": wlo,
            "wphi": wphi,
            "wplo": wplo,
            "wbias": wbias,
        }
        for i in range(N_CORES)
    ]
    res = run_bass_kernel_spmd(nc, in_maps, core_ids=list(range(N_CORES)))
    out = np.concatenate([res.results[i]["out"] for i in range(N_CORES)], axis=0)
    return out.astype(np.float32) / 1024.0


if __name__ == "__main__":
    rng = np.random.default_rng(0)
    X = rng.standard_normal((B, N, C), dtype=np.float32)
    W_qkv = rng.standard_normal((3 * C, C), dtype=np.float32) * C**-0.5
    W_proj = rng.standard_normal((C, C), dtype=np.float32) * C**-0.5
    b_proj = np.zeros(C, dtype=np.float32)
    out = kernel(X, W_qkv, W_proj, b_proj, 1)
    print(out.shape, out.dtype)


# revision 20
# speedup vs baseline: 1.3899x; 1.0156x over previous
"""Trainium2 Bass kernel for two-stage pooled-query attention.

Problem (hardcoded):
    B=32, N=577, C=1024, H=16 heads, d=64, pooled queries 8x8 (3x3 mean over
    24x24 grid of non-cls tokens).
    qkv = X @ W_qkv.T ; pool Xq -> Qp ; s1 = softmax(Qp*s @ K^T) @ V ;
    s2 = softmax(Xq*s @ Qp^T) @ s1 ; out = s2 @ W_proj.T + b_proj

Strategy: pure data-parallel over batch across 8 NeuronCores (4 batches per
core, no collectives). The big QKV GEMM runs fp8 DoubleRow (K=256 per
instruction at 0.5 cycles/row) with a 3-term hi/lo hybrid split to hold
accuracy:
  X @ W ~= X_hi@W_hi + X_lo@W_hi + X_hi@W_lo, each operand split on the host
  into an e4m3 value plus an e4m3 residual (W_qkv pre-scaled by 32 so its
  residual clears the fp8 subnormal floor; the 32 cancels through the pooled
  query scale and W_proj/32).

The attention middle runs bf16 with no PE transposes:
  - Stage 1 is computed transposed, s1T [n, 2q] = qkT_k.T @ QpBD, so the
    A1 weights land directly in the lhsT layout Qd needs. The softmax
    denominator Z1 comes from a 1-column ones matmul (~free), and 1/Z1 is
    folded per-q-partition into stage 2's weights instead of into Qd.
  - Stage 2 is computed transposed too, E2T [2q, n] = exp(QpBD.T @ qkT_q),
    the per-(token,head) denominator is produced pre-broadcast by one
    block-diag-ones matmul, and a single fused scalar_tensor_tensor gives
    e2n = E2T * r1[q] * (1/Z2) -- both normalizations in one op. The output
    outT [c, n] = QdBD_raw @ e2n needs no A2 transpose either.

Batches are software-pipelined: batch b+1's QKV GEMM instructions are
emitted interleaved into batch b's attention middle, so the in-order PE
queue always has independent work while the middle waits on softmax.
"""

import os
import sys

import numpy as np

sys.path.insert(0, "/opt/trn_rl_repo")

import ml_dtypes  # noqa: E402

import concourse.tile as tile  # noqa: E402
from concourse import bacc, mybir  # noqa: E402
from concourse.bass_utils import run_bass_kernel_spmd  # noqa: E402

B, N, C = 32, 577, 1024
H, D = 16, 64
SCALE = D ** -0.5
N_CORES = 8
NB = B // N_CORES  # batches per core

BF16 = mybir.dt.bfloat16
F32 = mybir.dt.float32
FP8 = mybir.dt.float8e4
DR = mybir.MatmulPerfMode.DoubleRow
WSCALE = 32.0  # host pre-scale on W_qkv^T (keeps fp8 residuals normal)
MULT = mybir.AluOpType.mult

# token chunks of 577 = 4*128 + 65
TOK = [(0, 128), (128, 128), (256, 128), (384, 128), (512, 65)]
# free-dim chunks of 577 for wide matmuls: one full psum bank + a stub
NF = [(0, 512), (512, 65)]
EXP = mybir.ActivationFunctionType.Exp


def build_program(nb: int = NB):
    nc = bacc.Bacc("TRN2", target_bir_lowering=False, debug=False)

    # host pre-arranges operands into the exact SBUF tile layout
    # [c2, p, i(sub-chunk), v(hi/lo), cols] so every DMA is contiguous
    x8_d = nc.dram_tensor("x8", [nb, 4, 128, 2, 2, 640], FP8, kind="ExternalInput")
    w8_d = nc.dram_tensor("w8", [4, 128, 2, 2, 3 * C], FP8, kind="ExternalInput")
    wp8_d = nc.dram_tensor("wp8", [4, 128, 2, 2, C], FP8, kind="ExternalInput")
    wbias_d = nc.dram_tensor("wbias", [1, C], BF16, kind="ExternalInput")
    out_d = nc.dram_tensor("out", [nb, N, C], BF16, kind="ExternalOutput")

    with tile.TileContext(nc) as tc:
        const_pool = tc.alloc_tile_pool(name="const", bufs=1)
        w_pool = tc.alloc_tile_pool(name="w", bufs=1)
        sb = tc.alloc_tile_pool(name="sb", bufs=2)
        ps_big = tc.alloc_tile_pool(name="ps_big", bufs=5, space="PSUM")
        ps_small = tc.alloc_tile_pool(name="ps_small", bufs=3, space="PSUM")

        ones = const_pool.tile([1, 128], BF16, tag="ones")
        nc.gpsimd.memset(ones[:], 1.0)
        ones128 = const_pool.tile([128, 1], BF16, tag="ones128")
        nc.gpsimd.memset(ones128[:], 1.0)
        # block-diag ones [2q, 128]: col j sums the q-rows of head(j)
        onesbd = const_pool.tile([128, 128], BF16, tag="onesbd")
        nc.gpsimd.memset(onesbd[:], 0.0)
        nc.gpsimd.memset(onesbd[0:64, 0:64], 1.0)
        nc.gpsimd.memset(onesbd[64:128, 64:128], 1.0)

        # first batch's X goes out before the (much larger) weight DMAs so
        # the QKV gemm can start immediately; weights stream behind. X and W
        # ship as fp8 hi/lo chunk-pair tiles [128, 2, cols] (dim 1 = the two
        # K=128 sub-chunks one DoubleRow matmul contracts).
        def load_x(b):
            tiles = []
            for c2 in range(4):
                x4 = sb.tile([128, 2, 2, 640], FP8, tag=f"x4{c2}", bufs=2)
                nc.sync.dma_start(x4[:], x8_d[b, c2])
                tiles.append((x4[:, :, 0, :], x4[:, :, 1, :]))
            return tiles

        XT0 = load_x(0)

        wh, wl = [], []
        w4s = []
        for c2 in range(4):
            w4 = w_pool.tile([128, 2, 2, 3 * C], FP8, tag=f"w4{c2}")
            w4s.append(w4)
            wh.append(w4[:, :, 0, :])
            wl.append(w4[:, :, 1, :])
        for blk in range(6):
            cs = slice(512 * blk, 512 * (blk + 1))
            for c2 in range(4):
                nc.sync.dma_start(w4s[c2][:, :, :, cs], w8_d[c2, :, :, :, cs])
        wph, wpl = [], []
        for c2 in range(4):
            t = w_pool.tile([128, 2, 2, C], FP8, tag=f"wp4{c2}")
            nc.sync.dma_start(t[:], wp8_d[c2])
            wph.append(t[:, :, 0, :])
            wpl.append(t[:, :, 1, :])
        wb = w_pool.tile([1, C], BF16, tag="wb")
        nc.sync.dma_start(wb[:], wbias_d[:])

        # persistent zeros: these tiles only ever get their "active" region
        # rewritten, so zero every rotation buffer once up front and never
        # memset in the loop.
        for _ in range(2):
            for j in range(8):
                qp = sb.tile([128, 128], BF16, tag=f"qp{j}")
                nc.gpsimd.memset(qp[:], 0.0)
            for pp in range(8):
                qd = sb.tile([128, 128], BF16, tag=f"qd{pp}")
                nc.gpsimd.memset(qd[:], 0.0)
        # (row 64 = token 576 is real and rewritten every batch; rows 65+ are
        # pad and must stay finite-zero. Partition ranges must start aligned,
        # so zero [64:128] once -- the loop re-writes row 64 before reading.)
        for _ in range(3):
            a1t = sb.tile([128, 5, 128], BF16, tag="a1t5", bufs=3)
            nc.gpsimd.memset(a1t[64:128, 4, :], 0.0)
        for _ in range(2):
            vt = sb.tile([128, C], BF16, tag="v4", bufs=2)
            nc.gpsimd.memset(vt[64:128, :], 0.0)

        # bias broadcast [128, 1024]; built lazily (first use is phase 8)
        bias = const_pool.tile([128, C], BF16, tag="bias")
        bias_built = [False]

        def build_bias():
            if bias_built[0]:
                return
            bias_built[0] = True
            for half in range(2):
                cs = slice(512 * half, 512 * (half + 1))
                bps = ps_big.tile([128, 512], F32, tag="pbig")
                nc.tensor.matmul(
                    bps[:], ones[0:1, :], wb[0:1, cs], start=True, stop=True
                )
                nc.any.tensor_copy(bias[:, cs], bps[:])

        # ---- front: QKV gemm + V + pooling for one batch, as thunks so the
        # emission can interleave into the previous batch's middle ----
        def front_thunks(b, XT):
            qkT = []
            V = []
            QpBD = []

            def phase2_chunk(cc):
                # q/k channels transposed: qkT [c, n] = 32 * true
                qt = sb.tile([128, 640], BF16, tag=f"qkt{cc}", bufs=2)
                csl = slice(128 * cc, 128 * (cc + 1))
                for ci, (n0, nw) in enumerate(NF):
                    ps = (ps_big if nw > 128 else ps_small).tile(
                        [128, nw], F32, tag="pbig" if nw > 128 else "psmall"
                    )
                    terms = (
                        [(wh[c2], XT[c2][0]) for c2 in range(4)]
                        + [(wl[c2], XT[c2][0]) for c2 in range(4)]
                        + [(wh[c2], XT[c2][1]) for c2 in range(4)]
                    )
                    for ti, (wt, xt_) in enumerate(terms):
                        nc.tensor.matmul(
                            ps[:],
                            wt[:, :, csl],
                            xt_[:, :, n0 : n0 + nw],
                            start=(ti == 0),
                            stop=(ti == 11),
                            perf_mode=DR,
                        )
                    if (cc + ci) % 2 == 0:
                        nc.vector.tensor_copy(qt[:, n0 : n0 + nw], ps[:])
                    else:
                        nc.scalar.copy(qt[:, n0 : n0 + nw], ps[:])
                qkT.append(qt)

            def phase3_tile(t):
                # V natural layout [n, c] = 32 * true; pad rows stay zero
                # because X pad columns are zero in both hi and lo.
                toff, rows = TOK[t]
                vt = sb.tile([128, C], BF16, tag=f"v{t}", bufs=2)
                lo = toff if rows == 128 else 512
                lw = 2 * rows
                for h2 in range(2):
                    ps = ps_big.tile([128, 512], F32, tag="pbig")
                    cs = slice(2048 + 512 * h2, 2048 + 512 * (h2 + 1))
                    terms = (
                        [(XT[c2][0], wh[c2]) for c2 in range(4)]
                        + [(XT[c2][0], wl[c2]) for c2 in range(4)]
                        + [(XT[c2][1], wh[c2]) for c2 in range(4)]
                    )
                    for ti, (xt_, wt) in enumerate(terms):
                        nc.tensor.matmul(
                            ps[0:rows, :],
                            xt_[:, :, lo : lo + rows],
                            wt[:, :, cs],
                            start=(ti == 0),
                            stop=(ti == 11),
                            perf_mode=DR,
                        )
                    nc.scalar.copy(vt[0:rows, 512 * h2 : 512 * (h2 + 1)], ps[0:rows, :])
                V.append(vt)

            def pool_j(j):
                # pooled queries, block-diag [c, 2q] per pair; qkT carries
                # 32x, so alpha makes qp = SCALE * Qp_true / 32 which renders
                # both score matmuls exact.
                qsum = sb.tile([128, 64], F32, tag="qsum", bufs=3)
                view = qkT[j][:, 0:576].rearrange(
                    "p (pr dr pc dc) -> p pr pc dr dc", pr=8, dr=3, pc=8, dc=3
                )
                nc.vector.reduce_sum(qsum[:], view, axis=mybir.AxisListType.XY)
                alpha = SCALE / (9.0 * WSCALE * WSCALE)
                qp = sb.tile([128, 128], BF16, tag=f"qp{j}")
                nc.gpsimd.tensor_scalar_mul(qp[0:64, 0:64], qsum[0:64, :], alpha)
                nc.gpsimd.tensor_scalar_mul(qp[64:128, 64:128], qsum[64:128, :], alpha)
                QpBD.append(qp)

            thunks = [lambda cc=cc: phase2_chunk(cc) for cc in range(16)]
            thunks += [lambda t=t: phase3_tile(t) for t in range(5)]
            thunks += [lambda j=j: pool_j(j) for j in range(8)]
            return thunks, (qkT, V, QpBD)

        # ---- back: attention middle + output projection for one batch ----
        def back_emit(b, state, filler):
            qkT, V, QpBD = state
            fill_i = [0]

            def fill(k):
                for _ in range(k):
                    if fill_i[0] < len(filler):
                        filler[fill_i[0]]()
                        fill_i[0] += 1

            A1T = [None] * 8
            E2 = [None] * 8
            RZ = [None] * 8
            R1 = [None] * 8
            E2N = [None] * 8
            QdBD = [None] * 8
            OTH = [None] * 4
            OTL = [None] * 4

            def scores(pp):
                # s1T [n, 2q]: lhsT = qkT_k chunk, rhs = QpBD; exp rows
                # limited to real tokens (pad rows stay prologue-zero)
                pa = ps_big.tile([128, 512], F32, tag="pbig")
                for t in range(4):
                    nc.tensor.matmul(
                        pa[:, 128 * t : 128 * (t + 1)],
                        qkT[8 + pp][:, 128 * t : 128 * (t + 1)],
                        QpBD[pp][:],
                        start=True,
                        stop=True,
                    )
                pb = ps_small.tile([128, 128], F32, tag="psmall")
                nc.tensor.matmul(
                    pb[:], qkT[8 + pp][:, 512:640], QpBD[pp][:], start=True, stop=True
                )
                a1t = sb.tile([128, 5, 128], BF16, tag="a1t5", bufs=3)
                nc.scalar.activation(
                    a1t[:, 0:4, :], pa[:].rearrange("p (t q) -> p t q", t=4), EXP
                )
                nc.scalar.activation(a1t[0:65, 4, :], pb[0:65, :], EXP)
                A1T[pp] = a1t

                # s2T [2q, n] = QpBD.T @ qkT_q; exp straight to E2 bf16
                e2 = sb.tile([128, 640], BF16, tag="e2", bufs=3)
                for n0, nw in NF:
                    ps = (ps_big if nw > 128 else ps_small).tile(
                        [128, nw], F32, tag="pbig" if nw > 128 else "psmall"
                    )
                    nc.tensor.matmul(
                        ps[:], QpBD[pp][:], qkT[pp][:, n0 : n0 + nw],
                        start=True, stop=True,
                    )
                    nc.scalar.activation(e2[:, n0 : n0 + nw], ps[:], EXP)
                E2[pp] = e2

            def tails(pp):
                a1t = A1T[pp]
                # Qd_raw pair product [2q, dv] plus a Z1 ones-column, both
                # accumulated into one psum bank; diag blocks kept raw
                # (1/Z1 is folded into e2n below)
                qd_ps = ps_small.tile([128, 129], F32, tag="psmall")
                for t in range(5):
                    nc.tensor.matmul(
                        qd_ps[:, 0:128],
                        a1t[:, t, :],
                        V[t][:, 128 * pp : 128 * (pp + 1)],
                        start=(t == 0),
                        stop=(t == 4),
                    )
                for t in range(5):
                    nc.tensor.matmul(
                        qd_ps[:, 128:129],
                        a1t[:, t, :],
                        ones128[:],
                        start=(t == 0),
                        stop=(t == 4),
                    )
                r1 = sb.tile([128, 1], F32, tag="r1", bufs=3)
                nc.vector.reciprocal(r1[:], qd_ps[:, 128:129])
                R1[pp] = r1
                qd = sb.tile([128, 128], BF16, tag=f"qd{pp}")
                nc.vector.tensor_copy(qd[0:64, 0:64], qd_ps[0:64, 0:64])
                nc.vector.tensor_copy(qd[64:128, 64:128], qd_ps[64:128, 64:128])
                QdBD[pp] = qd

                # Z2 pre-broadcast over q-rows, then 1/Z2
                e2 = E2[pp]
                rz = sb.tile([128, 640], F32, tag="rz", bufs=3)
                for n0, nw in NF:
                    zb = (ps_big if nw > 128 else ps_small).tile(
                        [128, nw], F32, tag="pbig" if nw > 128 else "psmall"
                    )
                    nc.tensor.matmul(
                        zb[:], onesbd[:], e2[:, n0 : n0 + nw], start=True, stop=True
                    )
                    nc.vector.reciprocal(rz[:, n0 : n0 + nw], zb[:])
                RZ[pp] = rz

                # fused: e2n = (E2 * r1[q-partition]) * (1/Z2)
                e2n = sb.tile([128, 640], BF16, tag="e2n", bufs=4)
                nc.vector.scalar_tensor_tensor(
                    e2n[:, 0:577], e2[:, 0:577], r1[:, 0:1], rz[:, 0:577],
                    op0=MULT, op1=MULT,
                )
                E2N[pp] = e2n

            def out_mm(pp):
                e2n = E2N[pp]
                oa = ps_big.tile([128, 512], F32, tag="pbig")
                ob = ps_small.tile([128, 65], F32, tag="psmall")
                nc.tensor.matmul(
                    oa[:], QdBD[pp][:], e2n[:, 0:512], start=True, stop=True
                )
                nc.tensor.matmul(
                    ob[:], QdBD[pp][:], e2n[:, 512:577], start=True, stop=True
                )
                c2, sub = pp // 2, pp % 2
                if sub == 0:
                    oth = sb.tile([128, 2, 640], FP8, tag=f"oth{c2}", bufs=2)
                    otl = sb.tile([128, 2, 640], FP8, tag=f"otl{c2}", bufs=2)
                    OTH[c2], OTL[c2] = oth, otl
                oth, otl = OTH[c2], OTL[c2]
                # hi = fp8(outT); lo = fp8(outT - hi), split across engines
                SUB = mybir.AluOpType.subtract
                nc.scalar.copy(oth[:, sub, 0:512], oa[:])
                nc.scalar.copy(oth[:, sub, 512:577], ob[:])
                nc.vector.tensor_tensor(
                    otl[:, sub, 0:512], oa[:], oth[:, sub, 0:512], op=SUB
                )
                nc.vector.tensor_tensor(
                    otl[:, sub, 512:577], ob[:], oth[:, sub, 512:577], op=SUB
                )

            for pp in range(8):
                scores(pp)
                if pp >= 1:
                    tails(pp - 1)
                if pp >= 3:
                    out_mm(pp - 3)
                fill(2)
            tails(7)
            out_mm(5)
            fill(1)
            out_mm(6)
            fill(1)
            out_mm(7)

            # ---- output projection + bias -> bf16 -> DRAM ----
            build_bias()
            for t, (toff, rows) in enumerate(TOK):
                lo = toff if rows == 128 else 512
                y = sb.tile([128, 1024], BF16, tag="y", bufs=2)
                for half in range(2):
                    cs = slice(512 * half, 512 * (half + 1))
                    ps = ps_big.tile([128, 512], F32, tag="pbig")
                    terms = (
                        [(OTH[c2], wph[c2]) for c2 in range(4)]
                        + [(OTL[c2], wph[c2]) for c2 in range(4)]
                        + [(OTH[c2], wpl[c2]) for c2 in range(4)]
                    )
                    for ti, (ot_, wt) in enumerate(terms):
                        nc.tensor.matmul(
                            ps[0:rows, :],
                            ot_[:, :, lo : lo + rows],
                            wt[:, :, cs],
                            start=(ti == 0),
                            stop=(ti == 11),
                            perf_mode=DR,
                        )
                    if (t + half) % 2 == 0:
                        nc.vector.tensor_tensor(
                            y[0:rows, cs], ps[0:rows, :], bias[0:rows, cs],
                            op=mybir.AluOpType.add,
                        )
                    else:
                        nc.scalar.activation(
                            y[0:rows, cs], ps[0:rows, :],
                            mybir.ActivationFunctionType.Copy,
                        )
                        nc.gpsimd.tensor_tensor(
                            y[0:rows, cs], y[0:rows, cs], bias[0:rows, cs],
                            op=mybir.AluOpType.add,
                        )
                    fill(1)
                nc.sync.dma_start(out_d[b, toff : toff + rows, :], y[0:rows, :])
            fill(100)

        repeat = int(os.environ.get("KERNEL_REPEAT", "1"))
        total = nb * repeat
        # emit batch 0's front directly; every later front interleaves into
        # the previous batch's middle as PE filler.
        thunks, state = front_thunks(0, XT0)
        for th in thunks:
            th()
        for i in range(total):
            b = i % nb
            if i + 1 < total:
                XTn = load_x((i + 1) % nb)
                nxt_thunks, nxt_state = front_thunks((i + 1) % nb, XTn)
            else:
                nxt_thunks, nxt_state = [], None
            back_emit(b, state, nxt_thunks)
            state = nxt_state

        for p in (ps_small, ps_big, sb, w_pool, const_pool):
            p.release()

    nc.compile()
    return nc


_NC_CACHE = {}


def _get_nc(nb: int = NB):
    if nb not in _NC_CACHE:
        _NC_CACHE[nb] = build_program(nb)
    return _NC_CACHE[nb]


def _split_fp8(a):
    """e4m3 value + e4m3 residual (round-to-nearest both times)."""
    hi = a.astype(ml_dtypes.float8_e4m3)
    lo = (a - hi.astype(np.float32)).astype(ml_dtypes.float8_e4m3)
    return hi, lo


def kernel(X, W_qkv, W_proj, b_proj, layer_idx=None):
    assert X.shape == (B, N, C)
    nc = _get_nc(NB)
    xt = np.zeros((B, C, 640), dtype=np.float32)
    xt[:, :, :N] = np.asarray(X, dtype=np.float32).transpose(0, 2, 1)
    xhi, xlo = _split_fp8(xt)
    # [b, c2, p, i, v, n]
    x8 = np.ascontiguousarray(
        np.stack([xhi, xlo], axis=2)
        .reshape(B, 4, 2, 128, 2, 640)
        .transpose(0, 1, 3, 2, 4, 5)
    )
    wq32 = np.ascontiguousarray(np.asarray(W_qkv, dtype=np.float32).T) * 32.0
    whi, wlo = _split_fp8(wq32)
    # [c2, p, i, v, c]
    w8 = np.ascontiguousarray(
        np.stack([whi, wlo], axis=1)
        .reshape(4, 2, 128, 2, 3 * C)
        .transpose(0, 2, 1, 3, 4)
    )
    wp32 = np.ascontiguousarray(np.asarray(W_proj, dtype=np.float32).T) * 32.0
    wphi, wplo = _split_fp8(wp32)
    wp8 = np.ascontiguousarray(
        np.stack([wphi, wplo], axis=1)
        .reshape(4, 2, 128, 2, C)
        .transpose(0, 2, 1, 3, 4)
    )
    wbias = (np.asarray(b_proj, dtype=np.float32).reshape(1, C) * 1024.0).astype(
        ml_dtypes.bfloat16
    )
    in_maps = [
        {
            "x8": x8[NB * i : NB * (i + 1)],
            "w8": w8,
            "wp8": wp8,
            "wbias": wbias,
        }
        for i in range(N_CORES)
    ]
    res = run_bass_kernel_spmd(nc, in_maps, core_ids=list(range(N_CORES)))
    out = np.concatenate([res.results[i]["out"] for i in range(N_CORES)], axis=0)
    return out.astype(np.float32) / 1024.0


if __name__ == "__main__":
    rng = np.random.default_rng(0)
    X = rng.standard_normal((B, N, C), dtype=np.float32)
    W_qkv = rng.standard_normal((3 * C, C), dtype=np.float32) * C**-0.5
    W_proj = rng.standard_normal((C, C), dtype=np.float32) * C**-0.5
    b_proj = np.zeros(C, dtype=np.float32)
    out = kernel(X, W_qkv, W_proj, b_proj, 1)
    print(out.shape, out.dtype)


# revision 22
# speedup vs baseline: 1.4182x; 1.0204x over previous
"""Trainium2 Bass kernel for two-stage pooled-query attention.

Problem (hardcoded):
    B=32, N=577, C=1024, H=16 heads, d=64, pooled queries 8x8 (3x3 mean over
    24x24 grid of non-cls tokens).
    qkv = X @ W_qkv.T ; pool Xq -> Qp ; s1 = softmax(Qp*s @ K^T) @ V ;
    s2 = softmax(Xq*s @ Qp^T) @ s1 ; out = s2 @ W_proj.T + b_proj

Strategy: pure data-parallel over batch across 8 NeuronCores (4 batches per
core, no collectives). The big QKV GEMM runs fp8 DoubleRow (K=256 per
instruction at 0.5 cycles/row) with a 3-term hi/lo hybrid split to hold
accuracy:
  X @ W ~= X_hi@W_hi + X_lo@W_hi + X_hi@W_lo, each operand split on the host
  into an e4m3 value plus an e4m3 residual (W_qkv pre-scaled by 32 so its
  residual clears the fp8 subnormal floor; the 32 cancels through the pooled
  query scale and W_proj/32).

The attention middle runs bf16 with no PE transposes:
  - Stage 1 is computed transposed, s1T [n, 2q] = qkT_k.T @ QpBD, so the
    A1 weights land directly in the lhsT layout Qd needs. The softmax
    denominator Z1 comes from a 1-column ones matmul (~free), and 1/Z1 is
    folded per-q-partition into stage 2's weights instead of into Qd.
  - Stage 2 is computed transposed too, E2T [2q, n] = exp(QpBD.T @ qkT_q),
    the per-(token,head) denominator is produced pre-broadcast by one
    block-diag-ones matmul, and a single fused scalar_tensor_tensor gives
    e2n = E2T * r1[q] * (1/Z2) -- both normalizations in one op. The output
    outT [c, n] = QdBD_raw @ e2n needs no A2 transpose either.

Batches are software-pipelined: batch b+1's QKV GEMM instructions are
emitted interleaved into batch b's attention middle, so the in-order PE
queue always has independent work while the middle waits on softmax.
"""

import os
import sys

import numpy as np

sys.path.insert(0, "/opt/trn_rl_repo")

import ml_dtypes  # noqa: E402

import concourse.tile as tile  # noqa: E402
from concourse import bacc, mybir  # noqa: E402
from concourse.bass_utils import run_bass_kernel_spmd  # noqa: E402

B, N, C = 32, 577, 1024
H, D = 16, 64
SCALE = D ** -0.5
N_CORES = 8
NB = B // N_CORES  # batches per core

BF16 = mybir.dt.bfloat16
F32 = mybir.dt.float32
FP8 = mybir.dt.float8e4
DR = mybir.MatmulPerfMode.DoubleRow
WSCALE = 32.0  # host pre-scale on W_qkv^T (keeps fp8 residuals normal)
MULT = mybir.AluOpType.mult

# token chunks of 577 = 4*128 + 65
TOK = [(0, 128), (128, 128), (256, 128), (384, 128), (512, 65)]
# free-dim chunks of 577 for wide matmuls: one full psum bank + a stub
NF = [(0, 512), (512, 65)]
EXP = mybir.ActivationFunctionType.Exp


def build_program(nb: int = NB):
    nc = bacc.Bacc("TRN2", target_bir_lowering=False, debug=False)

    # host pre-arranges operands into the exact SBUF tile layout
    # [c2, p, i(sub-chunk), v(hi/lo), cols] so every DMA is contiguous
    x8_d = nc.dram_tensor("x8", [nb, 4, 128, 2, 2, 640], FP8, kind="ExternalInput")
    w8_d = nc.dram_tensor("w8", [4, 128, 2, 2, 3 * C], FP8, kind="ExternalInput")
    wp8_d = nc.dram_tensor("wp8", [4, 128, 2, 2, C], FP8, kind="ExternalInput")
    wbias_d = nc.dram_tensor("wbias", [1, C], BF16, kind="ExternalInput")
    out_d = nc.dram_tensor("out", [nb, N, C], BF16, kind="ExternalOutput")

    with tile.TileContext(nc) as tc:
        const_pool = tc.alloc_tile_pool(name="const", bufs=1)
        w_pool = tc.alloc_tile_pool(name="w", bufs=1)
        sb = tc.alloc_tile_pool(name="sb", bufs=2)
        ps_big = tc.alloc_tile_pool(name="ps_big", bufs=5, space="PSUM")
        ps_small = tc.alloc_tile_pool(name="ps_small", bufs=3, space="PSUM")

        ones = const_pool.tile([1, 128], BF16, tag="ones")
        nc.gpsimd.memset(ones[:], 1.0)
        ones128 = const_pool.tile([128, 1], BF16, tag="ones128")
        nc.gpsimd.memset(ones128[:], 1.0)
        # block-diag ones [2q, 128]: col j sums the q-rows of head(j)
        onesbd = const_pool.tile([128, 128], BF16, tag="onesbd")
        nc.gpsimd.memset(onesbd[:], 0.0)
        nc.gpsimd.memset(onesbd[0:64, 0:64], 1.0)
        nc.gpsimd.memset(onesbd[64:128, 64:128], 1.0)

        # first batch's X goes out before the (much larger) weight DMAs so
        # the QKV gemm can start immediately; weights stream behind. X and W
        # ship as fp8 hi/lo chunk-pair tiles [128, 2, cols] (dim 1 = the two
        # K=128 sub-chunks one DoubleRow matmul contracts).
        def load_x(b):
            tiles = []
            for c2 in range(4):
                x4 = sb.tile([128, 2, 2, 640], FP8, tag=f"x4{c2}", bufs=2)
                nc.sync.dma_start(x4[:], x8_d[b, c2])
                tiles.append((x4[:, :, 0, :], x4[:, :, 1, :]))
            return tiles

        XT0 = load_x(0)

        wh, wl = [], []
        w4s = []
        for c2 in range(4):
            w4 = w_pool.tile([128, 2, 2, 3 * C], FP8, tag=f"w4{c2}")
            w4s.append(w4)
            wh.append(w4[:, :, 0, :])
            wl.append(w4[:, :, 1, :])
        for blk in range(6):
            cs = slice(512 * blk, 512 * (blk + 1))
            for c2 in range(4):
                nc.sync.dma_start(w4s[c2][:, :, :, cs], w8_d[c2, :, :, :, cs])
        wph, wpl = [], []
        for c2 in range(4):
            t = w_pool.tile([128, 2, 2, C], FP8, tag=f"wp4{c2}")
            nc.sync.dma_start(t[:], wp8_d[c2])
            wph.append(t[:, :, 0, :])
            wpl.append(t[:, :, 1, :])
        wb = w_pool.tile([1, C], BF16, tag="wb")
        nc.sync.dma_start(wb[:], wbias_d[:])

        # persistent zeros: these tiles only ever get their "active" region
        # rewritten, so zero every rotation buffer once up front and never
        # memset in the loop.
        for _ in range(2):
            for j in range(8):
                qp = sb.tile([128, 128], BF16, tag=f"qp{j}")
                nc.gpsimd.memset(qp[:], 0.0)
            for pp in range(8):
                qd = sb.tile([128, 128], BF16, tag=f"qd{pp}")
                nc.gpsimd.memset(qd[:], 0.0)
        # (row 64 = token 576 is real and rewritten every batch; rows 65+ are
        # pad and must stay finite-zero. Partition ranges must start aligned,
        # so zero [64:128] once -- the loop re-writes row 64 before reading.)
        for _ in range(3):
            a1t = sb.tile([128, 5, 128], BF16, tag="a1t5", bufs=3)
            nc.gpsimd.memset(a1t[64:128, 4, :], 0.0)
        for _ in range(2):
            vt = sb.tile([128, C], BF16, tag="v4", bufs=2)
            nc.gpsimd.memset(vt[64:128, :], 0.0)

        # bias broadcast [128, 1024]; built lazily (first use is phase 8)
        bias = const_pool.tile([128, C], BF16, tag="bias")
        bias_built = [False]

        def build_bias():
            if bias_built[0]:
                return
            bias_built[0] = True
            for half in range(2):
                cs = slice(512 * half, 512 * (half + 1))
                bps = ps_big.tile([128, 512], F32, tag="pbig")
                nc.tensor.matmul(
                    bps[:], ones[0:1, :], wb[0:1, cs], start=True, stop=True
                )
                nc.any.tensor_copy(bias[:, cs], bps[:])

        # ---- front: QKV gemm + V + pooling for one batch, as thunks so the
        # emission can interleave into the previous batch's middle ----
        def front_thunks(b, XT):
            qkT = []
            V = []
            QpBD = []

            def phase2_chunk(cc):
                # q/k channels transposed: qkT [c, n] = 32 * true
                qt = sb.tile([128, 640], BF16, tag=f"qkt{cc}", bufs=2)
                csl = slice(128 * cc, 128 * (cc + 1))
                for ci, (n0, nw) in enumerate(NF):
                    ps = (ps_big if nw > 128 else ps_small).tile(
                        [128, nw], F32, tag="pbig" if nw > 128 else "psmall"
                    )
                    terms = (
                        [(wh[c2], XT[c2][0]) for c2 in range(4)]
                        + [(wl[c2], XT[c2][0]) for c2 in range(4)]
                        + [(wh[c2], XT[c2][1]) for c2 in range(4)]
                    )
                    for ti, (wt, xt_) in enumerate(terms):
                        nc.tensor.matmul(
                            ps[:],
                            wt[:, :, csl],
                            xt_[:, :, n0 : n0 + nw],
                            start=(ti == 0),
                            stop=(ti == 11),
                            perf_mode=DR,
                        )
                    if (cc + ci) % 2 == 0:
                        nc.vector.tensor_copy(qt[:, n0 : n0 + nw], ps[:])
                    else:
                        nc.scalar.copy(qt[:, n0 : n0 + nw], ps[:])
                qkT.append(qt)

            def phase3_tile(t):
                # V natural layout [n, c] = 32 * true; pad rows stay zero
                # because X pad columns are zero in both hi and lo.
                toff, rows = TOK[t]
                vt = sb.tile([128, C], BF16, tag=f"v{t}", bufs=2)
                lo = toff if rows == 128 else 512
                lw = 2 * rows
                for h2 in range(2):
                    ps = ps_big.tile([128, 512], F32, tag="pbig")
                    cs = slice(2048 + 512 * h2, 2048 + 512 * (h2 + 1))
                    terms = (
                        [(XT[c2][0], wh[c2]) for c2 in range(4)]
                        + [(XT[c2][0], wl[c2]) for c2 in range(4)]
                        + [(XT[c2][1], wh[c2]) for c2 in range(4)]
                    )
                    for ti, (xt_, wt) in enumerate(terms):
                        nc.tensor.matmul(
                            ps[0:rows, :],
                            xt_[:, :, lo : lo + rows],
                            wt[:, :, cs],
                            start=(ti == 0),
                            stop=(ti == 11),
                            perf_mode=DR,
                        )
                    nc.scalar.copy(vt[0:rows, 512 * h2 : 512 * (h2 + 1)], ps[0:rows, :])
                V.append(vt)

            def pool_j(j):
                # pooled queries, block-diag [c, 2q] per pair; qkT carries
                # 32x, so alpha makes qp = SCALE * Qp_true / 32 which renders
                # both score matmuls exact.
                qsum = sb.tile([128, 64], F32, tag="qsum", bufs=3)
                view = qkT[j][:, 0:576].rearrange(
                    "p (pr dr pc dc) -> p pr pc dr dc", pr=8, dr=3, pc=8, dc=3
                )
                nc.vector.reduce_sum(qsum[:], view, axis=mybir.AxisListType.XY)
                alpha = SCALE / (9.0 * WSCALE * WSCALE)
                qp = sb.tile([128, 128], BF16, tag=f"qp{j}")
                nc.gpsimd.tensor_scalar_mul(qp[0:64, 0:64], qsum[0:64, :], alpha)
                nc.gpsimd.tensor_scalar_mul(qp[64:128, 64:128], qsum[64:128, :], alpha)
                QpBD.append(qp)

            thunks = [lambda cc=cc: phase2_chunk(cc) for cc in range(16)]
            thunks += [lambda t=t: phase3_tile(t) for t in range(5)]
            thunks += [lambda j=j: pool_j(j) for j in range(8)]
            return thunks, (qkT, V, QpBD)

        # ---- back: attention middle + output projection for one batch ----
        def back_emit(b, state, filler):
            qkT, V, QpBD = state
            fill_i = [0]

            def fill(k):
                for _ in range(k):
                    if fill_i[0] < len(filler):
                        filler[fill_i[0]]()
                        fill_i[0] += 1

            A1T = [None] * 8
            E2 = [None] * 8
            RZ = [None] * 8
            R1 = [None] * 8
            E2N = [None] * 8
            QdBD = [None] * 8
            OTH = [None] * 4
            OTL = [None] * 4

            def scores(pp):
                # s1T [n, 2q]: lhsT = qkT_k chunk, rhs = QpBD; exp rows
                # limited to real tokens (pad rows stay prologue-zero)
                pa = ps_big.tile([128, 512], F32, tag="pbig")
                for t in range(4):
                    nc.tensor.matmul(
                        pa[:, 128 * t : 128 * (t + 1)],
                        qkT[8 + pp][:, 128 * t : 128 * (t + 1)],
                        QpBD[pp][:],
                        start=True,
                        stop=True,
                    )
                pb = ps_small.tile([128, 128], F32, tag="psmall")
                nc.tensor.matmul(
                    pb[:], qkT[8 + pp][:, 512:640], QpBD[pp][:], start=True, stop=True
                )
                a1t = sb.tile([128, 5, 128], BF16, tag="a1t5", bufs=3)
                nc.scalar.activation(
                    a1t[:, 0:4, :], pa[:].rearrange("p (t q) -> p t q", t=4), EXP
                )
                nc.scalar.activation(a1t[0:65, 4, :], pb[0:65, :], EXP)
                A1T[pp] = a1t

                # s2T [2q, n] = QpBD.T @ qkT_q; exp straight to E2 bf16
                e2 = sb.tile([128, 640], BF16, tag="e2", bufs=3)
                for n0, nw in NF:
                    ps = (ps_big if nw > 128 else ps_small).tile(
                        [128, nw], F32, tag="pbig" if nw > 128 else "psmall"
                    )
                    nc.tensor.matmul(
                        ps[:], QpBD[pp][:], qkT[pp][:, n0 : n0 + nw],
                        start=True, stop=True,
                    )
                    nc.scalar.activation(e2[:, n0 : n0 + nw], ps[:], EXP)
                E2[pp] = e2

            def tails(pp):
                a1t = A1T[pp]
                # Qd_raw pair product [2q, dv] plus a Z1 ones-column, both
                # accumulated into one psum bank; diag blocks kept raw
                # (1/Z1 is folded into e2n below)
                qd_ps = ps_small.tile([128, 129], F32, tag="psmall")
                for t in range(5):
                    nc.tensor.matmul(
                        qd_ps[:, 0:128],
                        a1t[:, t, :],
                        V[t][:, 128 * pp : 128 * (pp + 1)],
                        start=(t == 0),
                        stop=(t == 4),
                    )
                for t in range(5):
                    nc.tensor.matmul(
                        qd_ps[:, 128:129],
                        a1t[:, t, :],
                        ones128[:],
                        start=(t == 0),
                        stop=(t == 4),
                    )
                r1 = sb.tile([128, 1], F32, tag="r1", bufs=3)
                nc.vector.reciprocal(r1[:], qd_ps[:, 128:129])
                R1[pp] = r1
                qd = sb.tile([128, 128], BF16, tag=f"qd{pp}")
                nc.vector.tensor_copy(qd[0:64, 0:64], qd_ps[0:64, 0:64])
                nc.vector.tensor_copy(qd[64:128, 64:128], qd_ps[64:128, 64:128])
                QdBD[pp] = qd

                # Z2 pre-broadcast over q-rows, then 1/Z2
                e2 = E2[pp]
                rz = sb.tile([128, 640], F32, tag="rz", bufs=3)
                for n0, nw in NF:
                    zb = (ps_big if nw > 128 else ps_small).tile(
                        [128, nw], F32, tag="pbig" if nw > 128 else "psmall"
                    )
                    nc.tensor.matmul(
                        zb[:], onesbd[:], e2[:, n0 : n0 + nw], start=True, stop=True
                    )
                    nc.vector.reciprocal(rz[:, n0 : n0 + nw], zb[:])
                RZ[pp] = rz

                # fused: e2n = (E2 * r1[q-partition]) * (1/Z2)
                e2n = sb.tile([128, 640], BF16, tag="e2n", bufs=4)
                nc.vector.scalar_tensor_tensor(
                    e2n[:, 0:577], e2[:, 0:577], r1[:, 0:1], rz[:, 0:577],
                    op0=MULT, op1=MULT,
                )
                E2N[pp] = e2n

            def out_mm(pp):
                e2n = E2N[pp]
                oa = ps_big.tile([128, 512], F32, tag="pbig")
                ob = ps_small.tile([128, 65], F32, tag="psmall")
                nc.tensor.matmul(
                    oa[:], QdBD[pp][:], e2n[:, 0:512], start=True, stop=True
                )
                nc.tensor.matmul(
                    ob[:], QdBD[pp][:], e2n[:, 512:577], start=True, stop=True
                )
                c2, sub = pp // 2, pp % 2
                if sub == 0:
                    oth = sb.tile([128, 2, 640], FP8, tag=f"oth{c2}", bufs=2)
                    otl = sb.tile([128, 2, 640], FP8, tag=f"otl{c2}", bufs=2)
                    OTH[c2], OTL[c2] = oth, otl
                oth, otl = OTH[c2], OTL[c2]
                # hi = fp8(outT); lo = fp8(outT - hi), split across engines
                SUB = mybir.AluOpType.subtract
                nc.scalar.copy(oth[:, sub, 0:512], oa[:])
                nc.scalar.copy(oth[:, sub, 512:577], ob[:])
                nc.vector.tensor_tensor(
                    otl[:, sub, 0:512], oa[:], oth[:, sub, 0:512], op=SUB
                )
                nc.vector.tensor_tensor(
                    otl[:, sub, 512:577], ob[:], oth[:, sub, 512:577], op=SUB
                )

            for pp in range(8):
                scores(pp)
                if pp >= 1:
                    tails(pp - 1)
                if pp >= 3:
                    out_mm(pp - 3)
                fill(2)
            tails(7)
            out_mm(5)
            fill(1)
            out_mm(6)
            fill(1)
            out_mm(7)
            fill(100)
            return OTH, OTL

        # ---- output projection + bias -> bf16 -> DRAM; emitted as thunks
        # and used as stall-free PE filler for the NEXT batch's middle ----
        def phase8_thunks(b, OTH, OTL):
            build_bias()
            Y = {}

            def unit(t, half):
                toff, rows = TOK[t]
                lo = toff if rows == 128 else 512
                if half == 0:
                    Y[t] = sb.tile([128, 1024], BF16, tag="y", bufs=2, name="y")
                y = Y[t]
                cs = slice(512 * half, 512 * (half + 1))
                ps = ps_big.tile([128, 512], F32, tag="pbig")
                terms = (
                    [(OTH[c2], wph[c2]) for c2 in range(4)]
                    + [(OTL[c2], wph[c2]) for c2 in range(4)]
                    + [(OTH[c2], wpl[c2]) for c2 in range(4)]
                )
                for ti, (ot_, wt) in enumerate(terms):
                    nc.tensor.matmul(
                        ps[0:rows, :],
                        ot_[:, :, lo : lo + rows],
                        wt[:, :, cs],
                        start=(ti == 0),
                        stop=(ti == 11),
                        perf_mode=DR,
                    )
                if (t + half) % 2 == 0:
                    nc.vector.tensor_tensor(
                        y[0:rows, cs], ps[0:rows, :], bias[0:rows, cs],
                        op=mybir.AluOpType.add,
                    )
                else:
                    nc.scalar.activation(
                        y[0:rows, cs], ps[0:rows, :],
                        mybir.ActivationFunctionType.Copy,
                    )
                    nc.gpsimd.tensor_tensor(
                        y[0:rows, cs], y[0:rows, cs], bias[0:rows, cs],
                        op=mybir.AluOpType.add,
                    )
                if half == 1:
                    nc.sync.dma_start(out_d[b, toff : toff + rows, :], y[0:rows, :])

            return [
                lambda t=t, half=half: unit(t, half)
                for t in range(5)
                for half in range(2)
            ]

        repeat = int(os.environ.get("KERNEL_REPEAT", "1"))
        total = nb * repeat
        # emit batch 0's front directly; every later front interleaves into
        # the previous batch's middle as PE filler, and every batch's output
        # projection interleaves into the NEXT batch's middle.
        thunks, state = front_thunks(0, XT0)
        for th in thunks:
            th()
        ph8 = []
        for i in range(total):
            b = i % nb
            if i + 1 < total:
                XTn = load_x((i + 1) % nb)
                nxt_thunks, nxt_state = front_thunks((i + 1) % nb, XTn)
            else:
                nxt_thunks, nxt_state = [], None
            oth, otl = back_emit(b, state, ph8 + nxt_thunks)
            ph8 = phase8_thunks(b, oth, otl)
            state = nxt_state
        for th in ph8:
            th()

        for p in (ps_small, ps_big, sb, w_pool, const_pool):
            p.release()

    nc.compile()
    return nc


_NC_CACHE = {}


def _get_nc(nb: int = NB):
    if nb not in _NC_CACHE:
        _NC_CACHE[nb] = build_program(nb)
    return _NC_CACHE[nb]


def _split_fp8(a):
    """e4m3 value + e4m3 residual (round-to-nearest both times)."""
    hi = a.astype(ml_dtypes.float8_e4m3)
    lo = (a - hi.astype(np.float32)).astype(ml_dtypes.float8_e4m3)
    return hi, lo


def kernel(X, W_qkv, W_proj, b_proj, layer_idx=None):
    assert X.shape == (B, N, C)
    nc = _get_nc(NB)
    xt = np.zeros((B, C, 640), dtype=np.float32)
    xt[:, :, :N] = np.asarray(X, dtype=np.float32).transpose(0, 2, 1)
    xhi, xlo = _split_fp8(xt)
    # [b, c2, p, i, v, n]
    x8 = np.ascontiguousarray(
        np.stack([xhi, xlo], axis=2)
        .reshape(B, 4, 2, 128, 2, 640)
        .transpose(0, 1, 3, 2, 4, 5)
    )
    wq32 = np.ascontiguousarray(np.asarray(W_qkv, dtype=np.float32).T) * 32.0
    whi, wlo = _split_fp8(wq32)
    # [c2, p, i, v, c]
    w8 = np.ascontiguousarray(
        np.stack([whi, wlo], axis=1)
        .reshape(4, 2, 128, 2, 3 * C)
        .transpose(0, 2, 1, 3, 4)
    )
    wp32 = np.ascontiguousarray(np.asarray(W_proj, dtype=np.float32).T) * 32.0
    wphi, wplo = _split_fp8(wp32)
    wp8 = np.ascontiguousarray(
        np.stack([wphi, wplo], axis=1)
        .reshape(4, 2, 128, 2, C)
        .transpose(0, 2, 1, 3, 4)
    )
    wbias = (np.asarray(b_proj, dtype=np.float32).reshape(1, C) * 1024.0).astype(
        ml_dtypes.bfloat16
    )
    in_maps = [
        {
            "x8": x8[NB * i : NB * (i + 1)],
            "w8": w8,
            "wp8": wp8,
            "wbias": wbias,
        }
        for i in range(N_CORES)
    ]
    res = run_bass_kernel_spmd(nc, in_maps, core_ids=list(range(N_CORES)))
    out = np.concatenate([res.results[i]["out"] for i in range(N_CORES)], axis=0)
    return out.astype(np.float32) / 1024.0


if __name__ == "__main__":
    rng = np.random.default_rng(0)
    X = rng.standard_normal((B, N, C), dtype=np.float32)
    W_qkv = rng.standard_normal((3 * C, C), dtype=np.float32) * C**-0.5
    W_proj = rng.standard_normal((C, C), dtype=np.float32) * C**-0.5
    b_proj = np.zeros(C, dtype=np.float32)
    out = kernel(X, W_qkv, W_proj, b_proj, 1)
    print(out.shape, out.dtype)
